# revision 40
# baseline (speedup 1.0000x reference)
"""Trainium2 Bass kernel for GQA attention prefill (nn_Attention).

Reference semantics (b=1, s=2048, dim=4096, 32 q heads, 8 kv heads, hd=128):
  xq = x @ wq.T ; xk = x @ wk.T ; xv = x @ wv.T
  xq, xk = rope(xq), rope(xk) ; xq, xk = rmsnorm(xq), rmsnorm(xk)
  o = softmax(q k^T / sqrt(hd) + mask) v          (grouped: 4 q heads / kv head)
  out = o @ wo.T

Sharding: tensor-parallel over heads on 8 cores — core c owns q heads
4c..4c+3 and kv head c; wo is sharded on its input dim; per-core partial
outputs are summed on the host.

Per-core pipeline (bf16 matmuls, fp32 accumulate):
  A: proj [s,e] -> rope+rmsnorm -> PE-transpose q,k to [hd,s].  The first
     attention block B(0) is interleaved into A's tail (its QK->exp->PV
     chains hide under A's dense projection stream).
  B: scores^T[sk,sq] = kT.T @ qT per 128-key chunk; causal mask applied as
     an additive -3000 identity-matmul into the scores psum (keeps the
     mask off the DVE critical path); exp on ACT; PV and a ones[128,128]
     denominator matmul accumulate in psum (the ones stationary broadcasts
     the denominator to all 128 rows, so normalization is just a DVE
     reciprocal+multiply, no partition_broadcast).  Heads run in pairs so
     two independent QK->exp->PV chains overlap.
  C: out[s,d] += oT.T @ wo, interleaved one pw-group at a time into the
     NEXT B block's kc loop (pw psum evacuated on ACT) so the PE never
     idles on exp latency or the normalization chains.
Causality: fully-masked blocks are skipped entirely.
Timing discipline: every cross-engine wait the PE can hit is covered by
independent queued matmul work (phase-C drains at pair boundaries).
"""

import math
import numpy as np
import ml_dtypes

import concourse.bass as bass
import concourse.tile as tile
from concourse import bacc, mybir, bass_isa
from concourse.bass import ts
from concourse.masks import make_identity
from concourse.bass_utils import run_bass_kernel_spmd

BF16 = mybir.dt.bfloat16
FP32 = mybir.dt.float32
FP8 = mybir.dt.float8e4

N_CORES = 8
S = 2048          # sequence
D = 4096          # model dim
HD = 128          # head dim
HQ = 4            # q heads per core
E = HQ * HD       # q out dim per core (512)
T = S // 128      # 16 s-tiles of 128
CQ = S // 512     # 4 sq chunks of 512
KC = S // 128     # 16 sk chunks of 128
DC = 32           # d chunks of 128
EPS = 1e-5
INV_SQRT_HD = 1.0 / math.sqrt(HD)
SQRT_HD = math.sqrt(HD)

_CACHE = {}


def _pin_act_tables():
    """Keep every ACT function in one table set so no per-tile table
    reloads are emitted (Exp/Ln/Square/Copy all live in
    natural_log_exp_and_others)."""
    import functools
    import concourse.hw_specs as hw_specs
    import concourse.bass_interp as bass_interp
    orig = hw_specs.get_activation_tables

    @functools.cache
    def patched(module_arch):
        tabs = orig(module_arch)
        keep = "natural_log_exp_and_others"
        if keep not in tabs:
            return tabs
        E = mybir.ActivationFunctionType
        mine = {f for f in (getattr(E, n, None) for n in
                            ("Exp", "Ln", "Square", "Copy", "Identity"))
                if f is not None} & tabs[keep]
        # preserve set order/indices (act_func_set_id is positional); just
        # make `keep` the only set containing the functions this kernel uses
        return {name: (fns if name == keep else fns - mine)
                for name, fns in tabs.items()}

    bacc.get_activation_tables = patched
    bass_interp.get_activation_tables = patched


def build_bass(pin_tables=True, repeat=1, phases="ABC", denom_pe=True,
               fp8_proj=False, no_exp=False, no_chain=False, no_pd=False,
               no_mask=False, b0_in_a=True, loop_drain=0):
    if pin_tables:
        _pin_act_tables()
    nc = bacc.Bacc("TRN2", target_bir_lowering=False, debug=False,
                   num_devices=N_CORES)

    xtt = nc.dram_tensor("xtt", [T, 128, DC, 128], BF16, kind="ExternalInput").ap()
    wqt = nc.dram_tensor("wqt", [128, DC, E], BF16, kind="ExternalInput").ap()
    wkvt = nc.dram_tensor("wkvt", [128, DC, 2 * HD], BF16, kind="ExternalInput").ap()
    wo = nc.dram_tensor("wo", [128, HQ, D], BF16, kind="ExternalInput").ap()
    mtd = nc.dram_tensor("mtd", [128, KC, 128], BF16, kind="ExternalInput").ap()
    cos = nc.dram_tensor("cos", [128, T, E], BF16, kind="ExternalInput").ap()
    sin = nc.dram_tensor("sin", [128, T, E], BF16, kind="ExternalInput").ap()
    out = nc.dram_tensor("out", [S, D], mybir.dt.float16, kind="ExternalOutput").ap()

    with tile.TileContext(nc) as tc:
        _emit(nc, tc, xtt, wqt, wkvt, wo, mtd, cos, sin, out, repeat=repeat,
              phases=phases, denom_pe=denom_pe, no_exp=no_exp,
              no_chain=no_chain, no_pd=no_pd, no_mask=no_mask,
              fp8_proj=fp8_proj, b0_in_a=b0_in_a, loop_drain=loop_drain)
    nc.compile()
    return nc


def _emit(nc, tc, xtt, wqt, wkvt, wo, mtd, cos, sin, out, repeat=1,
          phases="ABC", denom_pe=True, no_exp=False, no_chain=False,
          no_pd=False, no_mask=False, fp8_proj=False, b0_in_a=True,
          loop_drain=0):
    from contextlib import ExitStack
    ctx = ExitStack()
    with ctx:
        res = ctx.enter_context(tc.tile_pool(name="res", bufs=1))
        xp = ctx.enter_context(tc.tile_pool(name="xp", bufs=2))
        fq = ctx.enter_context(tc.tile_pool(name="fq", bufs=2))
        sml = ctx.enter_context(tc.tile_pool(name="sml", bufs=2))
        pbuf = ctx.enter_context(tc.tile_pool(name="pbuf", bufs=16))
        accp = ctx.enter_context(tc.tile_pool(name="accp", bufs=3))
        stg = ctx.enter_context(tc.tile_pool(name="stg", bufs=6))
        psum = ctx.enter_context(tc.tile_pool(name="psum", bufs=1, space="PSUM"))

        # resident tensors
        if not fp8_proj:
            wq_sb = res.tile([128, DC, E], BF16)
            wkv_sb = res.tile([128, DC, 2 * HD], BF16)
        wo_sb = res.tile([128, HQ, D], BF16)
        mtd_sb = res.tile([128, KC, 128], BF16)

        vsb = res.tile([128, T, HD], BF16)       # v, [s, hd] layout
        qkT = res.tile([128, 5, T, 128], BF16)   # slots 0-3: qT heads, 4: kT
        oT = res.tile([128, HQ, T, 128], BF16)   # o^T per head: [hd, s]

        ident = res.tile([128, 128], BF16)
        make_identity(nc, ident[:])
        epsb = res.tile([128, 1], FP32)
        nc.vector.memset(epsb[:], EPS)
        ones = res.tile([128, 128], BF16)
        nc.vector.memset(ones[:], 1.0)
        if no_exp:
            cpt = res.tile([128, 512], BF16)
            nc.vector.memset(cpt[:], 0.01)
        if fp8_proj:
            xc8 = res.tile([128, DC, 128], FP8)
            nc.vector.memset(xc8[:], 0.0)
            wq8 = res.tile([128, DC, E], FP8)
            nc.vector.memset(wq8[:], 0.0)
            wkv8 = res.tile([128, DC, 2 * HD], FP8)
            nc.vector.memset(wkv8[:], 0.0)

        AF = mybir.ActivationFunctionType
        MUL = mybir.AluOpType.mult
        ADD = mybir.AluOpType.add

        # psum budget (8 banks): mix 4 {psq,pskv in A; ps in B} +
        # aux 2 {ptr in A; pw in B/C-interleave} + po 2
        PA = dict(tag="mix", bufs=4)
        AUX = dict(tag="aux", bufs=2)
        PS = dict(tag="mix", bufs=4)
        PO = dict(tag="po", bufs=2)

        loop_ctx = tc.For_i(0, repeat, 1) if repeat > 1 else None
        if loop_ctx is not None:
            ctx.enter_context(loop_ctx)

        # ---------------- Phase A: proj + rope + rmsnorm + transposes
        # Software-pipelined: PE stream is [proj(m), transposes(m-1), ...] so
        # the DVE/ACT chain of tile m runs under proj(m+1)'s matmuls.
        def a_proj(m, first):
            xc = xp.tile([128, DC, 128], BF16, tag="xc", name=f"xc{m}")
            if m == 0:
                # first accumulation step's operands lead the DMA queue
                nc.sync.dma_start(xc[:, 0:8, :], xtt[m][:, 0:8, :])
                if not fp8_proj:
                    nc.sync.dma_start(wq_sb[:, 0:8, :], wqt[:, 0:8, :])
                for g in range(8, DC, 8):
                    nc.sync.dma_start(xc[:, g:g + 8, :], xtt[m][:, g:g + 8, :])
                    if not fp8_proj:
                        nc.sync.dma_start(wq_sb[:, g:g + 8, :],
                                          wqt[:, g:g + 8, :])
                if not fp8_proj:
                    for g in range(0, DC, 8):
                        nc.sync.dma_start(wkv_sb[:, g:g + 8, :],
                                          wkvt[:, g:g + 8, :])
            elif m < 2:
                for g in range(0, DC, 8):
                    nc.sync.dma_start(xc[:, g:g + 8, :], xtt[m][:, g:g + 8, :])
            else:
                nc.sync.dma_start(xc[:], xtt[m])
            cs = fq.tile([128, E], BF16, tag="cos", name=f"cs{m}")
            nc.sync.dma_start(cs[:], cos[:, m, :])
            sn = fq.tile([128, E], BF16, tag="sin", name=f"sn{m}")
            nc.sync.dma_start(sn[:], sin[:, m, :])
            if m == 0:
                nc.sync.dma_start(mtd_sb[:], mtd[:])
            if m == 2:
                nc.sync.dma_start(wo_sb[:], wo[:])

            psq = psum.tile([128, E], FP32, name=f"psq{m}", **PA)
            pskv = psum.tile([128, 2 * HD], FP32, name=f"pskv{m}", **PA)
            if fp8_proj:
                DR = mybir.MatmulPerfMode.DoubleRow
                # fp8_proj==2 mimics the 3-term residual split (3 passes)
                passes = 3 if fp8_proj == 2 else 1
                NP = DC // 2
                for p in range(passes):
                    for t in range(NP):
                        nc.tensor.matmul(psq[:], xc8[:, 2 * t:2 * t + 2, :],
                                         wq8[:, 2 * t:2 * t + 2, :],
                                         start=(p == 0 and t == 0),
                                         stop=(p == passes - 1 and t == NP - 1),
                                         perf_mode=DR)
                for p in range(passes):
                    for t in range(NP):
                        nc.tensor.matmul(pskv[:], xc8[:, 2 * t:2 * t + 2, :],
                                         wkv8[:, 2 * t:2 * t + 2, :],
                                         start=(p == 0 and t == 0),
                                         stop=(p == passes - 1 and t == NP - 1),
                                         perf_mode=DR)
            else:
                for c in range(DC):
                    nc.tensor.matmul(psq[:], xc[:, c, :], wq_sb[:, c, :],
                                     start=(c == 0), stop=(c == DC - 1))
                for c in range(DC):
                    nc.tensor.matmul(pskv[:], xc[:, c, :], wkv_sb[:, c, :],
                                     start=(c == 0), stop=(c == DC - 1))

            # evacuate psum to sbuf bf16 (ACT), then rope on DVE in 2x mode
            qsb = sml.tile([128, E], BF16, tag="qsb", name=f"qsb{m}")
            nc.scalar.copy(qsb[:], psq[:])
            kvsb = sml.tile([128, 2 * HD], BF16, tag="kvsb", name=f"kvsb{m}")
            nc.scalar.copy(kvsb[:], pskv[:])
            nc.vector.tensor_copy(vsb[:, m, :], kvsb[:, HD:2 * HD])

            # rope(q): qro = q*cos + swap(q)*sin_signed
            tco = sml.tile([128, E], BF16, tag="tco", name=f"tco{m}")
            nc.vector.tensor_tensor(tco[:], qsb[:], cs[:], op=MUL)
            tro = sml.tile([128, E], BF16, tag="tro", name=f"tro{m}")
            q3 = qsb[:].rearrange("p (x two) -> p x two", two=2)
            t3 = tro[:].rearrange("p (x two) -> p x two", two=2)
            nc.vector.tensor_copy(t3[:, :, 0], q3[:, :, 1])
            nc.vector.tensor_copy(t3[:, :, 1], q3[:, :, 0])
            qro = sml.tile([128, E], BF16, tag="qro", name=f"qro{m}")
            nc.vector.tensor_tensor(tro[:], tro[:], sn[:], op=MUL)
            nc.vector.tensor_tensor(qro[:], tco[:], tro[:], op=ADD)

            # rope(k)
            tck = sml.tile([128, E], BF16, tag="tco", name=f"tck{m}")
            nc.vector.tensor_tensor(tck[:, 0:HD], kvsb[:, 0:HD], cs[:, 0:HD], op=MUL)
            trk = sml.tile([128, E], BF16, tag="tro", name=f"trk{m}")
            k3 = kvsb[:, 0:HD].rearrange("p (x two) -> p x two", two=2)
            r3 = trk[:, 0:HD].rearrange("p (x two) -> p x two", two=2)
            nc.vector.tensor_copy(r3[:, :, 0], k3[:, :, 1])
            nc.vector.tensor_copy(r3[:, :, 1], k3[:, :, 0])
            kro = sml.tile([128, E], BF16, tag="qro", name=f"kro{m}")
            nc.vector.tensor_tensor(trk[:, 0:HD], trk[:, 0:HD], sn[:, 0:HD], op=MUL)
            nc.vector.tensor_tensor(kro[:, 0:HD], tck[:, 0:HD], trk[:, 0:HD], op=ADD)

            # rmsnorm: rinv = exp(-0.5*ln(mean(t^2)+eps)), all in one ACT set
            sqs = sml.tile([128, 5], FP32, tag="sqs", name=f"sqs{m}")
            scr = sml.tile([128, HD], FP32, tag="scr", name=f"scr{m}")
            for h in range(HQ):
                nc.scalar.activation(scr[:], qro[:, ts(h, HD)], AF.Square,
                                     accum_out=sqs[:, h:h + 1])
            nc.scalar.activation(scr[:], kro[:, 0:HD], AF.Square,
                                 accum_out=sqs[:, 4:5])
            rin = sml.tile([128, 5], FP32, tag="rin", name=f"rin{m}")
            nc.scalar.activation(rin[:], sqs[:], AF.Ln, scale=1.0 / HD, bias=epsb[:])
            nc.scalar.activation(rin[:], rin[:], AF.Exp, scale=-0.5)

            qnt = sml.tile([128, E], BF16, tag="qnt", bufs=3, name=f"qnt{m}")
            knt = sml.tile([128, HD], BF16, tag="knt", bufs=3, name=f"knt{m}")
            for h in range(HQ):
                nc.scalar.mul(qnt[:, ts(h, HD)], qro[:, ts(h, HD)], rin[:, h:h + 1])
            nc.scalar.mul(knt[:], kro[:, 0:HD], rin[:, 4:5])
            return qnt, knt

        def a_post(m, qnt, knt):
            # transposes packed into one psum bank, single evac
            ptr = psum.tile([128, 5, 128], BF16, name=f"ptr{m}", **AUX)
            for h in range(HQ):
                nc.tensor.transpose(ptr[:, h, :], qnt[:, ts(h, HD)], ident[:])
            nc.tensor.transpose(ptr[:, 4, :], knt[:], ident[:])
            nc.scalar.copy(qkT[:, :, m, :], ptr[:])

        # ---- B(0) emitted as closures interleaved into phase A's tail:
        # its QK->exp->PV chains hide completely under A's dense projection
        # stream (deps: qkT/vsb for kc 0-3 are ready after a_post(3), i.e.
        # from m-slot 5 on).
        b0_in_a = (b0_in_a and "B" in phases and denom_pe
                   and not (no_exp or no_chain or no_pd))
        b0q = []
        if b0_in_a:
            b0state = {}

            def b0_qk(h, kc):
                j0 = 128 * kc
                ps = psum.tile([128, 512], FP32, name=f"b0sc{h}_{kc}", **PS)
                nc.tensor.matmul(ps[:, j0:], qkT[:, 4, kc, :],
                                 qkT[:, h, kc:4, :],
                                 start=True, stop=False,
                                 skip_group_check=True)
                j1 = min(j0 + 128, 512)
                nc.tensor.matmul(ps[:, j0:j1], ident[:], mtd_sb[:, kc, :],
                                 start=False, stop=True,
                                 skip_group_check=True)
                pt = pbuf.tile([128, 512], BF16, tag="pt",
                               name=f"b0pt{h}_{kc}")
                nc.scalar.activation(pt[:, j0:], ps[:, j0:], AF.Exp,
                                     scale=INV_SQRT_HD)
                b0state[(h, kc)] = pt

            def b0_pv(h, kc):
                j0 = 128 * kc
                pt = b0state.pop((h, kc))
                nc.tensor.matmul(b0state[("po", h)][:, j0:], vsb[:, kc, :],
                                 pt[:, j0:], start=(kc == 0), stop=(kc == 3))
                nc.tensor.matmul(b0state[("pd", h)][:, j0:], ones[:],
                                 pt[:, j0:], start=(kc == 0), stop=(kc == 3))

            def mk_b0_steps(h):
                def s0():
                    b0state[("po", h)] = psum.tile(
                        [128, 512], FP32, name=f"b0po{h}", **PO)
                    b0state[("pd", h)] = psum.tile(
                        [128, 512], FP32, name=f"b0pd{h}", **PO)
                    b0_qk(h, 0)
                    b0_qk(h, 1)
                    b0_pv(h, 0)

                def s1():
                    b0_qk(h, 2)
                    b0_pv(h, 1)

                def s2():
                    b0_qk(h, 3)
                    b0_pv(h, 2)

                def s3():
                    b0_pv(h, 3)

                def s4():
                    po_ = b0state.pop(("po", h))
                    pd_ = b0state.pop(("pd", h))
                    rb = accp.tile([128, 512], FP32, tag="rb",
                                   name=f"b0rb{h}")
                    nc.vector.reciprocal(rb[:], pd_[:])
                    nc.vector.tensor_tensor(oT[:, h, 0:4, :], po_[:], rb[:],
                                            op=MUL)
                return [s0, s1, s2, s3, s4]

            for h in range(HQ):
                b0q.extend(mk_b0_steps(h))

        prev = None
        for m in range(T):
            qk = a_proj(m, first=(m == 0))
            if prev is not None:
                a_post(m - 1, *prev)
            prev = qk
            if m >= 5:
                for _ in range(2):
                    if b0q:
                        b0q.pop(0)()
        a_post(T - 1, *prev)
        while b0q:
            b0q.pop(0)()

        if "B" not in phases:
            so = stg.tile([128, 512], mybir.dt.float16, tag="so", name="dummy")
            nc.vector.tensor_copy(so[:], qkT[:, 0, 0:4, :])
            nc.sync.dma_start(out[0:128, 0:512], so[:])
            return

        # ---------------- Phase B + interleaved C.
        # Heads run in PAIRS (two independent QK->exp->PV chains hide the
        # ACT exp latency from the PE), and phase C of block cq-1 is
        # interleaved one pw-group at a time into B(cq)'s kc loop so the PE
        # never idles waiting on ACT.
        def c_group(m, dc):
            pw = psum.tile([128, 512], FP32, name=f"pw{m}_{dc}", **AUX)
            for j in range(HQ):
                nc.tensor.matmul(pw[:], oT[:, j, m, :],
                                 wo_sb[:, j, ts(dc, 512)],
                                 start=(j == 0), stop=(j == HQ - 1))
            so = stg.tile([128, 512], mybir.dt.float16, tag="so",
                          name=f"so{m}_{dc}")
            # psum evacuation on ACT: DVE must stay clear for the
            # normalization chains that gate the po/pd psum rings
            nc.scalar.copy(so[:], pw[:])
            nc.sync.dma_start(out[ts(m, 128), ts(dc, 512)], so[:])

        pending = []  # deferred phase-C (m, dc) groups from the previous cq
        if b0_in_a and "C" in phases:
            pending = [(m, dc) for m in range(4) for dc in range(8)]

        def drain(n):
            for _ in range(min(n, len(pending))):
                m, dc = pending.pop(0)
                c_group(m, dc)

        for cq in range(1 if b0_in_a else 0, CQ):
            nkc = 4 * cq + 4
            for hp in range(HQ // 2):
                h0, h1 = 2 * hp, 2 * hp + 1
                po0 = psum.tile([128, 512], FP32, name=f"po{cq}_{h0}", **PO)
                po1 = psum.tile([128, 512], FP32, name=f"po{cq}_{h1}", **PO)
                if denom_pe:
                    pd0 = psum.tile([128, 512], FP32, name=f"pd{cq}_{h0}",
                                    tag="mix", bufs=4)
                    pd1 = psum.tile([128, 512], FP32, name=f"pd{cq}_{h1}",
                                    tag="mix", bufs=4)
                else:
                    # denominator accumulators on DVE (quad-tree over probs
                    # tiles; fp16 intermediates, fp32 acc) keep the PE free
                    acc0 = accp.tile([128, 512], FP32, tag="acc", bufs=2,
                                     name=f"acc{cq}_{h0}")
                    acc1 = accp.tile([128, 512], FP32, tag="acc", bufs=2,
                                     name=f"acc{cq}_{h1}")

                def qk_exp(h, kc):
                    # columns below j0 are fully masked (causal): skip them
                    j0 = max(0, 128 * (kc - 4 * cq))
                    ps = psum.tile([128, 512], FP32,
                                   name=f"sc{cq}_{h}_{kc}", **PS)
                    d0 = j0 // 128
                    nc.tensor.matmul(ps[:, j0:], qkT[:, 4, kc, :],
                                     qkT[:, h, 4 * cq + d0:4 * cq + 4, :],
                                     start=True, stop=(kc < 4 * cq),
                                     skip_group_check=(kc >= 4 * cq))
                    if kc >= 4 * cq and not no_mask:
                        # diagonal block: add -3000 to masked score positions
                        # (PE identity-matmul) so exp() yields exact zeros --
                        # keeps the mask off the DVE critical path
                        j1 = min(j0 + 128, 512)
                        nc.tensor.matmul(ps[:, j0:j1], ident[:],
                                         mtd_sb[:, kc, :],
                                         start=False, stop=True,
                                         skip_group_check=True)
                    if no_exp:
                        return cpt
                    pt = pbuf.tile([128, 512], BF16, tag="pt",
                                   name=f"pt{cq}_{h}_{kc}")
                    nc.scalar.activation(pt[:, j0:], ps[:, j0:], AF.Exp,
                                         scale=INV_SQRT_HD)
                    if j0 > 0 and not denom_pe:
                        # zero fully-masked columns so the denominator tree
                        # can sum full-width tiles
                        nc.gpsimd.memset(pt[:, 0:j0], 0.0)
                    return pt

                def pv(po, pd, kc, pt):
                    j0 = max(0, 128 * (kc - 4 * cq))
                    nc.tensor.matmul(po[:, j0:], vsb[:, kc, :], pt[:, j0:],
                                     start=(kc == 0), stop=(kc == nkc - 1))
                    if denom_pe:
                        nc.tensor.matmul(pd[:, j0:], ones[:], pt[:, j0:],
                                         start=(kc == 0),
                                         stop=(kc == nkc - 1))

                FP16 = mybir.dt.float16

                def den_quad(acc, g, pts4):
                    # acc (+)= pts4[0]+pts4[1]+pts4[2]+pts4[3], on DVE
                    t1 = accp.tile([128, 512], FP16, tag="t1", bufs=2,
                                   name=f"t1_{cq}_{g}")
                    t2 = accp.tile([128, 512], FP16, tag="t2", bufs=2,
                                   name=f"t2_{cq}_{g}")
                    nc.vector.tensor_tensor(t1[:], pts4[0][:], pts4[1][:],
                                            op=ADD)
                    nc.vector.tensor_tensor(t2[:], pts4[2][:], pts4[3][:],
                                            op=ADD)
                    if g == 0:
                        nc.vector.tensor_tensor(acc[:], t1[:], t2[:], op=ADD)
                    else:
                        nc.vector.tensor_tensor(t1[:], t1[:], t2[:], op=ADD)
                        nc.vector.tensor_tensor(acc[:], acc[:], t1[:], op=ADD)

                pts0 = [qk_exp(h0, 0)]
                pts1 = [qk_exp(h1, 0)]
                for kc in range(nkc):
                    if kc + 1 < nkc:
                        pts0.append(qk_exp(h0, kc + 1))
                    pv(po0, pd0 if denom_pe else None, kc, pts0[kc])
                    if kc + 1 < nkc:
                        pts1.append(qk_exp(h1, kc + 1))
                    pv(po1, pd1 if denom_pe else None, kc, pts1[kc])
                    if loop_drain and kc % 2 == 1:
                        drain(loop_drain)
                    if kc % 4 == 3 and not denom_pe:
                        den_quad(acc0, kc // 4, pts0[kc - 3:kc + 1])
                        den_quad(acc1, kc // 4, pts1[kc - 3:kc + 1])

                # cover the denominator/normalization chain (and the po
                # psum-ring reuse it gates) with independent phase-C work
                drain(6)
                if not no_chain:
                    if denom_pe:
                        for h, po_, pd_ in ((h0, po0, pd0), (h1, po1, pd1)):
                            rb = accp.tile([128, 512], FP32, tag="rb",
                                           name=f"rb{cq}_{h}")
                            nc.vector.reciprocal(rb[:], pd_[:])
                            nc.vector.tensor_tensor(oT[:, h, ts(cq, 4), :],
                                                    po_[:], rb[:], op=MUL)
                    else:
                        for h, po_, acc_ in ((h0, po0, acc0), (h1, po1, acc1)):
                            dn = accp.tile([128, 512], FP32, tag="dn", bufs=2,
                                           name=f"dn{cq}_{h}")
                            nc.gpsimd.partition_all_reduce(
                                dn[:], acc_[:], channels=128,
                                reduce_op=bass_isa.ReduceOp.add)
                            rb = accp.tile([128, 512], FP32, tag="rb",
                                           name=f"rb{cq}_{h}")
                            nc.vector.reciprocal(rb[:], dn[:])
                            nc.vector.tensor_tensor(oT[:, h, ts(cq, 4), :],
                                                    po_[:], rb[:], op=MUL)

            if "C" in phases:
                drain(len(pending))  # anything not yet drained
                pending = [(m, dc) for m in range(4 * cq, 4 * cq + 4)
                           for dc in range(8)]

        if "C" in phases:
            drain(len(pending))
        elif "B" in phases:
            so = stg.tile([128, 512], mybir.dt.float16, tag="so", name="dummyC")
            src = qkT[:, 0, 0:4, :] if no_chain else oT[:, 0, 0:4, :]
            nc.vector.tensor_copy(so[:], src)
            nc.sync.dma_start(out[0:128, 0:512], so[:])


def _prep_inputs(x, wq, wk, wv, wo, freqs_cis, mask):
    """Host-side shard + retile. Returns list of per-core input dicts."""
    bf = ml_dtypes.bfloat16
    x2 = np.asarray(x, dtype=np.float32).reshape(S, D)
    # xtt[m, p, c, s'] = x[128m+s', 128c+p]
    xtt = np.ascontiguousarray(
        x2.reshape(T, 128, DC, 128).transpose(0, 3, 2, 1)).astype(bf)

    fr = np.asarray(freqs_cis, dtype=np.float32)[..., 0]   # [S, 64]
    fi = np.asarray(freqs_cis, dtype=np.float32)[..., 1]
    cos_il = np.repeat(fr, 2, axis=1)                       # [S, 128]
    sin_il = np.repeat(fi, 2, axis=1)
    sin_il[:, 0::2] *= -1.0                                 # signed for swap-form
    cos_rep = np.tile(cos_il, (1, HQ))                      # [S, 512]
    sin_rep = np.tile(sin_il, (1, HQ))
    cos_t = np.ascontiguousarray(
        cos_rep.reshape(T, 128, E).transpose(1, 0, 2)).astype(bf)
    sin_t = np.ascontiguousarray(
        sin_rep.reshape(T, 128, E).transpose(1, 0, 2)).astype(bf)

    m2 = np.asarray(mask, dtype=np.float32)
    # per sk-chunk kc: the 128-wide partially-masked column block of the
    # transposed mask, as 0/1 visibility. mtd_t[p, kc, j] for global key
    # row 128*kc+p, query col 512*cq + 128*(kc-4*cq) + j.
    mtd_t = np.zeros((128, KC, 128), dtype=np.float32)
    for kc in range(KC):
        cq = kc // 4
        j0 = 128 * (kc - 4 * cq)
        qlo = 512 * cq + j0
        blk = m2[qlo:qlo + 128, 128 * kc:128 * (kc + 1)].T  # [sk 128, sq 128]
        mtd_t[:, kc, :] = np.where(blk > -1e29, 0.0, -3000.0)
    mtd_t = np.ascontiguousarray(mtd_t).astype(bf)

    wqf = np.asarray(wq, dtype=np.float32)
    wkf = np.asarray(wk, dtype=np.float32)
    wvf = np.asarray(wv, dtype=np.float32)
    wof = np.asarray(wo, dtype=np.float32)

    in_maps = []
    for c in range(N_CORES):
        wq_c = wqf[E * c:E * (c + 1), :]                    # [512, D]
        wqt = np.ascontiguousarray(
            wq_c.T.reshape(DC, 128, E).transpose(1, 0, 2)).astype(bf)
        wk_c = wkf[HD * c:HD * (c + 1), :]                  # [128, D]
        wv_c = wvf[HD * c:HD * (c + 1), :]
        wkv_c = np.concatenate([wk_c, wv_c], axis=0)        # [256, D]
        wkvt = np.ascontiguousarray(
            wkv_c.T.reshape(DC, 128, 2 * HD).transpose(1, 0, 2)).astype(bf)
        wo_c = wof[:, E * c:E * (c + 1)].T                  # [512 e, D]
        wo_t = np.ascontiguousarray(
            wo_c.reshape(HQ, 128, D).transpose(1, 0, 2)).astype(bf)
        in_maps.append({
            "xtt": xtt, "wqt": wqt, "wkvt": wkvt, "wo": wo_t,
            "mtd": mtd_t, "cos": cos_t, "sin": sin_t,
        })
    return in_maps


def kernel(x, wq, wk, wv, wo, freqs_cis, mask, start_pos=0):
    if "nc" not in _CACHE:
        _CACHE["nc"] = build_bass()
    nc = _CACHE["nc"]
    in_maps = _prep_inputs(x, wq, wk, wv, wo, freqs_cis, mask)
    res = run_bass_kernel_spmd(nc, in_maps, list(range(N_CORES)))
    total = np.zeros((S, D), dtype=np.float32)
    for c in range(N_CORES):
        total += res.results[c]["out"].astype(np.float32)
    return total.reshape(1, S, D)



# revision 43
# speedup vs baseline: 1.0069x; 1.0069x over previous
"""Trainium2 Bass kernel for GQA attention prefill (nn_Attention).

Reference semantics (b=1, s=2048, dim=4096, 32 q heads, 8 kv heads, hd=128):
  xq = x @ wq.T ; xk = x @ wk.T ; xv = x @ wv.T
  xq, xk = rope(xq), rope(xk) ; xq, xk = rmsnorm(xq), rmsnorm(xk)
  o = softmax(q k^T / sqrt(hd) + mask) v          (grouped: 4 q heads / kv head)
  out = o @ wo.T

Sharding: tensor-parallel over heads on 8 cores — core c owns q heads
4c..4c+3 and kv head c; wo is sharded on its input dim; per-core partial
outputs are summed on the host.

Per-core pipeline (bf16 matmuls, fp32 accumulate):
  A: proj [s,e] -> rope+rmsnorm -> PE-transpose q,k to [hd,s].  The first
     attention block B(0) is interleaved into A's tail (its QK->exp->PV
     chains hide under A's dense projection stream).
  B: scores^T[sk,sq] = kT.T @ qT per 128-key chunk; causal mask applied as
     an additive -3000 identity-matmul into the scores psum (keeps the
     mask off the DVE critical path); exp on ACT; PV and a ones[128,128]
     denominator matmul accumulate in psum (the ones stationary broadcasts
     the denominator to all 128 rows, so normalization is just a DVE
     reciprocal+multiply, no partition_broadcast).  Heads run in pairs so
     two independent QK->exp->PV chains overlap.
  C: out[s,d] += oT.T @ wo, interleaved one pw-group at a time into the
     NEXT B block's kc loop (pw psum evacuated on ACT) so the PE never
     idles on exp latency or the normalization chains.
Causality: fully-masked blocks are skipped entirely.
Timing discipline: every cross-engine wait the PE can hit is covered by
independent queued matmul work (phase-C drains at pair boundaries).
"""

import math
import numpy as np
import ml_dtypes

import concourse.bass as bass
import concourse.tile as tile
from concourse import bacc, mybir, bass_isa
from concourse.bass import ts
from concourse.masks import make_identity
from concourse.bass_utils import run_bass_kernel_spmd

BF16 = mybir.dt.bfloat16
FP32 = mybir.dt.float32
FP8 = mybir.dt.float8e4

N_CORES = 8
S = 2048          # sequence
D = 4096          # model dim
HD = 128          # head dim
HQ = 4            # q heads per core
E = HQ * HD       # q out dim per core (512)
T = S // 128      # 16 s-tiles of 128
CQ = S // 512     # 4 sq chunks of 512
KC = S // 128     # 16 sk chunks of 128
DC = 32           # d chunks of 128
EPS = 1e-5
INV_SQRT_HD = 1.0 / math.sqrt(HD)
SQRT_HD = math.sqrt(HD)

_CACHE = {}


def _pin_act_tables():
    """Keep every ACT function in one table set so no per-tile table
    reloads are emitted (Exp/Ln/Square/Copy all live in
    natural_log_exp_and_others)."""
    import functools
    import concourse.hw_specs as hw_specs
    import concourse.bass_interp as bass_interp
    orig = hw_specs.get_activation_tables

    @functools.cache
    def patched(module_arch):
        tabs = orig(module_arch)
        keep = "natural_log_exp_and_others"
        if keep not in tabs:
            return tabs
        E = mybir.ActivationFunctionType
        mine = {f for f in (getattr(E, n, None) for n in
                            ("Exp", "Ln", "Square", "Copy", "Identity"))
                if f is not None} & tabs[keep]
        # preserve set order/indices (act_func_set_id is positional); just
        # make `keep` the only set containing the functions this kernel uses
        return {name: (fns if name == keep else fns - mine)
                for name, fns in tabs.items()}

    bacc.get_activation_tables = patched
    bass_interp.get_activation_tables = patched


def build_bass(pin_tables=True, repeat=1, phases="ABC", denom_pe=True,
               fp8_proj=False, no_exp=False, no_chain=False, no_pd=False,
               no_mask=False, b0_in_a=True, loop_drain=0, dump_alt=False):
    if pin_tables:
        _pin_act_tables()
    nc = bacc.Bacc("TRN2", target_bir_lowering=False, debug=False,
                   num_devices=N_CORES)

    xtt = nc.dram_tensor("xtt", [T, 128, DC, 128], BF16, kind="ExternalInput").ap()
    wqt = nc.dram_tensor("wqt", [128, DC, E], BF16, kind="ExternalInput").ap()
    wkvt = nc.dram_tensor("wkvt", [128, DC, 2 * HD], BF16, kind="ExternalInput").ap()
    wo = nc.dram_tensor("wo", [128, HQ, D], BF16, kind="ExternalInput").ap()
    mtd = nc.dram_tensor("mtd", [128, KC, 128], BF16, kind="ExternalInput").ap()
    cos = nc.dram_tensor("cos", [128, T, E], BF16, kind="ExternalInput").ap()
    sin = nc.dram_tensor("sin", [128, T, E], BF16, kind="ExternalInput").ap()
    out = nc.dram_tensor("out", [S, D], mybir.dt.float16, kind="ExternalOutput").ap()

    with tile.TileContext(nc) as tc:
        _emit(nc, tc, xtt, wqt, wkvt, wo, mtd, cos, sin, out, repeat=repeat,
              phases=phases, denom_pe=denom_pe, no_exp=no_exp,
              no_chain=no_chain, no_pd=no_pd, no_mask=no_mask,
              fp8_proj=fp8_proj, b0_in_a=b0_in_a, loop_drain=loop_drain,
              dump_alt=dump_alt)
    nc.compile()
    return nc


def _emit(nc, tc, xtt, wqt, wkvt, wo, mtd, cos, sin, out, repeat=1,
          phases="ABC", denom_pe=True, no_exp=False, no_chain=False,
          no_pd=False, no_mask=False, fp8_proj=False, b0_in_a=True,
          loop_drain=0, dump_alt=False):
    from contextlib import ExitStack
    ctx = ExitStack()
    with ctx:
        res = ctx.enter_context(tc.tile_pool(name="res", bufs=1))
        xp = ctx.enter_context(tc.tile_pool(name="xp", bufs=2))
        fq = ctx.enter_context(tc.tile_pool(name="fq", bufs=2))
        sml = ctx.enter_context(tc.tile_pool(name="sml", bufs=2))
        pbuf = ctx.enter_context(tc.tile_pool(name="pbuf", bufs=16))
        accp = ctx.enter_context(tc.tile_pool(name="accp", bufs=3))
        stg = ctx.enter_context(tc.tile_pool(name="stg", bufs=6))
        psum = ctx.enter_context(tc.tile_pool(name="psum", bufs=1, space="PSUM"))

        # resident tensors
        if not fp8_proj:
            wq_sb = res.tile([128, DC, E], BF16)
            wkv_sb = res.tile([128, DC, 2 * HD], BF16)
        wo_sb = res.tile([128, HQ, D], BF16)
        mtd_sb = res.tile([128, KC, 128], BF16)

        vsb = res.tile([128, T, HD], BF16)       # v, [s, hd] layout
        qkT = res.tile([128, 5, T, 128], BF16)   # slots 0-3: qT heads, 4: kT
        oT = res.tile([128, HQ, T, 128], BF16)   # o^T per head: [hd, s]

        ident = res.tile([128, 128], BF16)
        make_identity(nc, ident[:])
        epsb = res.tile([128, 1], FP32)
        nc.vector.memset(epsb[:], EPS)
        ones = res.tile([128, 128], BF16)
        nc.vector.memset(ones[:], 1.0)
        if no_exp:
            cpt = res.tile([128, 512], BF16)
            nc.vector.memset(cpt[:], 0.01)
        if fp8_proj:
            xc8 = res.tile([128, DC, 128], FP8)
            nc.vector.memset(xc8[:], 0.0)
            wq8 = res.tile([128, DC, E], FP8)
            nc.vector.memset(wq8[:], 0.0)
            wkv8 = res.tile([128, DC, 2 * HD], FP8)
            nc.vector.memset(wkv8[:], 0.0)

        AF = mybir.ActivationFunctionType
        MUL = mybir.AluOpType.mult
        ADD = mybir.AluOpType.add

        # psum budget (8 banks): mix 4 {psq,pskv in A; ps in B} +
        # aux 2 {ptr in A; pw in B/C-interleave} + po 2
        PA = dict(tag="mix", bufs=4)
        AUX = dict(tag="aux", bufs=2)
        PS = dict(tag="mix", bufs=4)
        PO = dict(tag="po", bufs=2)

        loop_ctx = tc.For_i(0, repeat, 1) if repeat > 1 else None
        if loop_ctx is not None:
            ctx.enter_context(loop_ctx)

        # ---------------- Phase A: proj + rope + rmsnorm + transposes
        # Software-pipelined: PE stream is [proj(m), transposes(m-1), ...] so
        # the DVE/ACT chain of tile m runs under proj(m+1)'s matmuls.
        def a_proj(m, first):
            xc = xp.tile([128, DC, 128], BF16, tag="xc", name=f"xc{m}")
            if m == 0:
                # first accumulation step's operands lead the DMA queue
                nc.sync.dma_start(xc[:, 0:8, :], xtt[m][:, 0:8, :])
                if not fp8_proj:
                    nc.sync.dma_start(wq_sb[:, 0:8, :], wqt[:, 0:8, :])
                for g in range(8, DC, 8):
                    nc.sync.dma_start(xc[:, g:g + 8, :], xtt[m][:, g:g + 8, :])
                    if not fp8_proj:
                        nc.sync.dma_start(wq_sb[:, g:g + 8, :],
                                          wqt[:, g:g + 8, :])
                if not fp8_proj:
                    for g in range(0, DC, 8):
                        nc.sync.dma_start(wkv_sb[:, g:g + 8, :],
                                          wkvt[:, g:g + 8, :])
            elif m < 2:
                for g in range(0, DC, 8):
                    nc.sync.dma_start(xc[:, g:g + 8, :], xtt[m][:, g:g + 8, :])
            else:
                nc.sync.dma_start(xc[:], xtt[m])
            cs = fq.tile([128, E], BF16, tag="cos", name=f"cs{m}")
            nc.sync.dma_start(cs[:], cos[:, m, :])
            sn = fq.tile([128, E], BF16, tag="sin", name=f"sn{m}")
            nc.sync.dma_start(sn[:], sin[:, m, :])
            if m == 0:
                nc.sync.dma_start(mtd_sb[:], mtd[:])
            if m == 2:
                nc.sync.dma_start(wo_sb[:], wo[:])

            psq = psum.tile([128, E], FP32, name=f"psq{m}", **PA)
            pskv = psum.tile([128, 2 * HD], FP32, name=f"pskv{m}", **PA)
            if fp8_proj:
                DR = mybir.MatmulPerfMode.DoubleRow
                # fp8_proj==2 mimics the 3-term residual split (3 passes)
                passes = 3 if fp8_proj == 2 else 1
                NP = DC // 2
                for p in range(passes):
                    for t in range(NP):
                        nc.tensor.matmul(psq[:], xc8[:, 2 * t:2 * t + 2, :],
                                         wq8[:, 2 * t:2 * t + 2, :],
                                         start=(p == 0 and t == 0),
                                         stop=(p == passes - 1 and t == NP - 1),
                                         perf_mode=DR)
                for p in range(passes):
                    for t in range(NP):
                        nc.tensor.matmul(pskv[:], xc8[:, 2 * t:2 * t + 2, :],
                                         wkv8[:, 2 * t:2 * t + 2, :],
                                         start=(p == 0 and t == 0),
                                         stop=(p == passes - 1 and t == NP - 1),
                                         perf_mode=DR)
            else:
                for c in range(DC):
                    nc.tensor.matmul(psq[:], xc[:, c, :], wq_sb[:, c, :],
                                     start=(c == 0), stop=(c == DC - 1))
                for c in range(DC):
                    nc.tensor.matmul(pskv[:], xc[:, c, :], wkv_sb[:, c, :],
                                     start=(c == 0), stop=(c == DC - 1))

            # evacuate psum to sbuf bf16 (ACT), then rope on DVE in 2x mode
            qsb = sml.tile([128, E], BF16, tag="qsb", name=f"qsb{m}")
            nc.scalar.copy(qsb[:], psq[:])
            kvsb = sml.tile([128, 2 * HD], BF16, tag="kvsb", name=f"kvsb{m}")
            nc.scalar.copy(kvsb[:], pskv[:])
            nc.vector.tensor_copy(vsb[:, m, :], kvsb[:, HD:2 * HD])

            # rope(q): qro = q*cos + swap(q)*sin_signed
            tco = sml.tile([128, E], BF16, tag="tco", name=f"tco{m}")
            nc.vector.tensor_tensor(tco[:], qsb[:], cs[:], op=MUL)
            tro = sml.tile([128, E], BF16, tag="tro", name=f"tro{m}")
            q3 = qsb[:].rearrange("p (x two) -> p x two", two=2)
            t3 = tro[:].rearrange("p (x two) -> p x two", two=2)
            nc.vector.tensor_copy(t3[:, :, 0], q3[:, :, 1])
            nc.vector.tensor_copy(t3[:, :, 1], q3[:, :, 0])
            qro = sml.tile([128, E], BF16, tag="qro", name=f"qro{m}")
            nc.vector.tensor_tensor(tro[:], tro[:], sn[:], op=MUL)
            nc.vector.tensor_tensor(qro[:], tco[:], tro[:], op=ADD)

            # rope(k)
            tck = sml.tile([128, E], BF16, tag="tco", name=f"tck{m}")
            nc.vector.tensor_tensor(tck[:, 0:HD], kvsb[:, 0:HD], cs[:, 0:HD], op=MUL)
            trk = sml.tile([128, E], BF16, tag="tro", name=f"trk{m}")
            k3 = kvsb[:, 0:HD].rearrange("p (x two) -> p x two", two=2)
            r3 = trk[:, 0:HD].rearrange("p (x two) -> p x two", two=2)
            nc.vector.tensor_copy(r3[:, :, 0], k3[:, :, 1])
            nc.vector.tensor_copy(r3[:, :, 1], k3[:, :, 0])
            kro = sml.tile([128, E], BF16, tag="qro", name=f"kro{m}")
            nc.vector.tensor_tensor(trk[:, 0:HD], trk[:, 0:HD], sn[:, 0:HD], op=MUL)
            nc.vector.tensor_tensor(kro[:, 0:HD], tck[:, 0:HD], trk[:, 0:HD], op=ADD)

            # rmsnorm: rinv = exp(-0.5*ln(mean(t^2)+eps)), all in one ACT set
            sqs = sml.tile([128, 5], FP32, tag="sqs", name=f"sqs{m}")
            scr = sml.tile([128, HD], FP32, tag="scr", name=f"scr{m}")
            for h in range(HQ):
                nc.scalar.activation(scr[:], qro[:, ts(h, HD)], AF.Square,
                                     accum_out=sqs[:, h:h + 1])
            nc.scalar.activation(scr[:], kro[:, 0:HD], AF.Square,
                                 accum_out=sqs[:, 4:5])
            rin = sml.tile([128, 5], FP32, tag="rin", name=f"rin{m}")
            nc.scalar.activation(rin[:], sqs[:], AF.Ln, scale=1.0 / HD, bias=epsb[:])
            nc.scalar.activation(rin[:], rin[:], AF.Exp, scale=-0.5)

            qnt = sml.tile([128, E], BF16, tag="qnt", bufs=3, name=f"qnt{m}")
            knt = sml.tile([128, HD], BF16, tag="knt", bufs=3, name=f"knt{m}")
            for h in range(HQ):
                nc.scalar.mul(qnt[:, ts(h, HD)], qro[:, ts(h, HD)], rin[:, h:h + 1])
            nc.scalar.mul(knt[:], kro[:, 0:HD], rin[:, 4:5])
            return qnt, knt

        def a_post(m, qnt, knt):
            # transposes packed into one psum bank, single evac
            ptr = psum.tile([128, 5, 128], BF16, name=f"ptr{m}", **AUX)
            for h in range(HQ):
                nc.tensor.transpose(ptr[:, h, :], qnt[:, ts(h, HD)], ident[:])
            nc.tensor.transpose(ptr[:, 4, :], knt[:], ident[:])
            nc.scalar.copy(qkT[:, :, m, :], ptr[:])

        # ---- B(0) emitted as closures interleaved into phase A's tail:
        # its QK->exp->PV chains hide completely under A's dense projection
        # stream (deps: qkT/vsb for kc 0-3 are ready after a_post(3), i.e.
        # from m-slot 5 on).
        b0_in_a = (b0_in_a and "B" in phases and denom_pe
                   and not (no_exp or no_chain or no_pd))
        b0q = []
        if b0_in_a:
            b0state = {}

            def b0_qk(h, kc):
                j0 = 128 * kc
                ps = psum.tile([128, 512], FP32, name=f"b0sc{h}_{kc}", **PS)
                nc.tensor.matmul(ps[:, j0:], qkT[:, 4, kc, :],
                                 qkT[:, h, kc:4, :],
                                 start=True, stop=False,
                                 skip_group_check=True)
                j1 = min(j0 + 128, 512)
                nc.tensor.matmul(ps[:, j0:j1], ident[:], mtd_sb[:, kc, :],
                                 start=False, stop=True,
                                 skip_group_check=True)
                pt = pbuf.tile([128, 512], BF16, tag="pt",
                               name=f"b0pt{h}_{kc}")
                nc.scalar.activation(pt[:, j0:], ps[:, j0:], AF.Exp,
                                     scale=INV_SQRT_HD)
                b0state[(h, kc)] = pt

            def b0_pv(h, kc):
                j0 = 128 * kc
                pt = b0state.pop((h, kc))
                nc.tensor.matmul(b0state[("po", h)][:, j0:], vsb[:, kc, :],
                                 pt[:, j0:], start=(kc == 0), stop=(kc == 3))
                nc.tensor.matmul(b0state[("pd", h)][:, j0:], ones[:],
                                 pt[:, j0:], start=(kc == 0), stop=(kc == 3))

            def mk_b0_steps(h):
                def s0():
                    b0state[("po", h)] = psum.tile(
                        [128, 512], FP32, name=f"b0po{h}", **PO)
                    b0state[("pd", h)] = psum.tile(
                        [128, 512], FP32, name=f"b0pd{h}", **PO)
                    b0_qk(h, 0)
                    b0_qk(h, 1)
                    b0_pv(h, 0)

                def s1():
                    b0_qk(h, 2)
                    b0_pv(h, 1)

                def s2():
                    b0_qk(h, 3)
                    b0_pv(h, 2)

                def s3():
                    b0_pv(h, 3)

                def s4():
                    po_ = b0state.pop(("po", h))
                    pd_ = b0state.pop(("pd", h))
                    rb = accp.tile([128, 512], FP32, tag="rb",
                                   name=f"b0rb{h}")
                    nc.vector.reciprocal(rb[:], pd_[:])
                    nc.vector.tensor_tensor(oT[:, h, 0:4, :], po_[:], rb[:],
                                            op=MUL)
                return [s0, s1, s2, s3, s4]

            for h in range(HQ):
                b0q.extend(mk_b0_steps(h))

        prev = None
        for m in range(T):
            qk = a_proj(m, first=(m == 0))
            if prev is not None:
                a_post(m - 1, *prev)
            prev = qk
            if m >= 5:
                for _ in range(2):
                    if b0q:
                        b0q.pop(0)()
        a_post(T - 1, *prev)
        while b0q:
            b0q.pop(0)()

        if "B" not in phases:
            so = stg.tile([128, 512], mybir.dt.float16, tag="so", name="dummy")
            nc.vector.tensor_copy(so[:], qkT[:, 0, 0:4, :])
            nc.sync.dma_start(out[0:128, 0:512], so[:])
            return

        # ---------------- Phase B + interleaved C.
        # Heads run in PAIRS (two independent QK->exp->PV chains hide the
        # ACT exp latency from the PE), and phase C of block cq-1 is
        # interleaved one pw-group at a time into B(cq)'s kc loop so the PE
        # never idles waiting on ACT.
        def c_group(m, dc, alt=False):
            pw = psum.tile([128, 512], FP32, name=f"pw{m}_{dc}", **AUX)
            for j in range(HQ):
                nc.tensor.matmul(pw[:], oT[:, j, m, :],
                                 wo_sb[:, j, ts(dc, 512)],
                                 start=(j == 0), stop=(j == HQ - 1))
            so = stg.tile([128, 512], mybir.dt.float16, tag="so",
                          name=f"so{m}_{dc}")
            # psum evacuation on ACT by default: DVE must stay clear for
            # the normalization chains that gate the po/pd psum rings.
            # In bulk dumps (no chains in flight) alternate with DVE so
            # the aux-ring turnaround is not ACT-paced.
            if alt and dc % 2 == 1:
                nc.vector.tensor_copy(so[:], pw[:])
            else:
                nc.scalar.copy(so[:], pw[:])
            nc.sync.dma_start(out[ts(m, 128), ts(dc, 512)], so[:])

        pending = []  # deferred phase-C (m, dc) groups from the previous cq
        if b0_in_a and "C" in phases:
            pending = [(m, dc) for m in range(4) for dc in range(8)]

        def drain(n, alt=False):
            for _ in range(min(n, len(pending))):
                m, dc = pending.pop(0)
                c_group(m, dc, alt=alt)

        for cq in range(1 if b0_in_a else 0, CQ):
            nkc = 4 * cq + 4
            for hp in range(HQ // 2):
                h0, h1 = 2 * hp, 2 * hp + 1
                po0 = psum.tile([128, 512], FP32, name=f"po{cq}_{h0}", **PO)
                po1 = psum.tile([128, 512], FP32, name=f"po{cq}_{h1}", **PO)
                if denom_pe:
                    pd0 = psum.tile([128, 512], FP32, name=f"pd{cq}_{h0}",
                                    tag="mix", bufs=4)
                    pd1 = psum.tile([128, 512], FP32, name=f"pd{cq}_{h1}",
                                    tag="mix", bufs=4)
                else:
                    # denominator accumulators on DVE (quad-tree over probs
                    # tiles; fp16 intermediates, fp32 acc) keep the PE free
                    acc0 = accp.tile([128, 512], FP32, tag="acc", bufs=2,
                                     name=f"acc{cq}_{h0}")
                    acc1 = accp.tile([128, 512], FP32, tag="acc", bufs=2,
                                     name=f"acc{cq}_{h1}")

                def qk_exp(h, kc):
                    # columns below j0 are fully masked (causal): skip them
                    j0 = max(0, 128 * (kc - 4 * cq))
                    ps = psum.tile([128, 512], FP32,
                                   name=f"sc{cq}_{h}_{kc}", **PS)
                    d0 = j0 // 128
                    nc.tensor.matmul(ps[:, j0:], qkT[:, 4, kc, :],
                                     qkT[:, h, 4 * cq + d0:4 * cq + 4, :],
                                     start=True, stop=(kc < 4 * cq),
                                     skip_group_check=(kc >= 4 * cq))
                    if kc >= 4 * cq and not no_mask:
                        # diagonal block: add -3000 to masked score positions
                        # (PE identity-matmul) so exp() yields exact zeros --
                        # keeps the mask off the DVE critical path
                        j1 = min(j0 + 128, 512)
                        nc.tensor.matmul(ps[:, j0:j1], ident[:],
                                         mtd_sb[:, kc, :],
                                         start=False, stop=True,
                                         skip_group_check=True)
                    if no_exp:
                        return cpt
                    pt = pbuf.tile([128, 512], BF16, tag="pt",
                                   name=f"pt{cq}_{h}_{kc}")
                    nc.scalar.activation(pt[:, j0:], ps[:, j0:], AF.Exp,
                                         scale=INV_SQRT_HD)
                    if j0 > 0 and not denom_pe:
                        # zero fully-masked columns so the denominator tree
                        # can sum full-width tiles
                        nc.gpsimd.memset(pt[:, 0:j0], 0.0)
                    return pt

                def pv(po, pd, kc, pt):
                    j0 = max(0, 128 * (kc - 4 * cq))
                    nc.tensor.matmul(po[:, j0:], vsb[:, kc, :], pt[:, j0:],
                                     start=(kc == 0), stop=(kc == nkc - 1))
                    if denom_pe:
                        nc.tensor.matmul(pd[:, j0:], ones[:], pt[:, j0:],
                                         start=(kc == 0),
                                         stop=(kc == nkc - 1))

                FP16 = mybir.dt.float16

                def den_quad(acc, g, pts4):
                    # acc (+)= pts4[0]+pts4[1]+pts4[2]+pts4[3], on DVE
                    t1 = accp.tile([128, 512], FP16, tag="t1", bufs=2,
                                   name=f"t1_{cq}_{g}")
                    t2 = accp.tile([128, 512], FP16, tag="t2", bufs=2,
                                   name=f"t2_{cq}_{g}")
                    nc.vector.tensor_tensor(t1[:], pts4[0][:], pts4[1][:],
                                            op=ADD)
                    nc.vector.tensor_tensor(t2[:], pts4[2][:], pts4[3][:],
                                            op=ADD)
                    if g == 0:
                        nc.vector.tensor_tensor(acc[:], t1[:], t2[:], op=ADD)
                    else:
                        nc.vector.tensor_tensor(t1[:], t1[:], t2[:], op=ADD)
                        nc.vector.tensor_tensor(acc[:], acc[:], t1[:], op=ADD)

                pts0 = [qk_exp(h0, 0)]
                pts1 = [qk_exp(h1, 0)]
                for kc in range(nkc):
                    if kc + 1 < nkc:
                        pts0.append(qk_exp(h0, kc + 1))
                    pv(po0, pd0 if denom_pe else None, kc, pts0[kc])
                    if kc + 1 < nkc:
                        pts1.append(qk_exp(h1, kc + 1))
                    pv(po1, pd1 if denom_pe else None, kc, pts1[kc])
                    if loop_drain and kc % 2 == 1:
                        drain(loop_drain)
                    if kc % 4 == 3 and not denom_pe:
                        den_quad(acc0, kc // 4, pts0[kc - 3:kc + 1])
                        den_quad(acc1, kc // 4, pts1[kc - 3:kc + 1])

                # cover the denominator/normalization chain (and the po
                # psum-ring reuse it gates) with independent phase-C work
                drain(6)
                if not no_chain:
                    if denom_pe:
                        for h, po_, pd_ in ((h0, po0, pd0), (h1, po1, pd1)):
                            rb = accp.tile([128, 512], FP32, tag="rb",
                                           name=f"rb{cq}_{h}")
                            nc.vector.reciprocal(rb[:], pd_[:])
                            nc.vector.tensor_tensor(oT[:, h, ts(cq, 4), :],
                                                    po_[:], rb[:], op=MUL)
                    else:
                        for h, po_, acc_ in ((h0, po0, acc0), (h1, po1, acc1)):
                            dn = accp.tile([128, 512], FP32, tag="dn", bufs=2,
                                           name=f"dn{cq}_{h}")
                            nc.gpsimd.partition_all_reduce(
                                dn[:], acc_[:], channels=128,
                                reduce_op=bass_isa.ReduceOp.add)
                            rb = accp.tile([128, 512], FP32, tag="rb",
                                           name=f"rb{cq}_{h}")
                            nc.vector.reciprocal(rb[:], dn[:])
                            nc.vector.tensor_tensor(oT[:, h, ts(cq, 4), :],
                                                    po_[:], rb[:], op=MUL)

            if "C" in phases:
                # bulk dump: chains already issued, DVE free
                drain(len(pending), alt=dump_alt)
                pending = [(m, dc) for m in range(4 * cq, 4 * cq + 4)
                           for dc in range(8)]

        if "C" in phases:
            drain(len(pending), alt=dump_alt)
        elif "B" in phases:
            so = stg.tile([128, 512], mybir.dt.float16, tag="so", name="dummyC")
            src = qkT[:, 0, 0:4, :] if no_chain else oT[:, 0, 0:4, :]
            nc.vector.tensor_copy(so[:], src)
            nc.sync.dma_start(out[0:128, 0:512], so[:])


def _prep_inputs(x, wq, wk, wv, wo, freqs_cis, mask):
    """Host-side shard + retile. Returns list of per-core input dicts."""
    bf = ml_dtypes.bfloat16
    x2 = np.asarray(x, dtype=np.float32).reshape(S, D)
    # xtt[m, p, c, s'] = x[128m+s', 128c+p]
    xtt = np.ascontiguousarray(
        x2.reshape(T, 128, DC, 128).transpose(0, 3, 2, 1)).astype(bf)

    fr = np.asarray(freqs_cis, dtype=np.float32)[..., 0]   # [S, 64]
    fi = np.asarray(freqs_cis, dtype=np.float32)[..., 1]
    cos_il = np.repeat(fr, 2, axis=1)                       # [S, 128]
    sin_il = np.repeat(fi, 2, axis=1)
    sin_il[:, 0::2] *= -1.0                                 # signed for swap-form
    cos_rep = np.tile(cos_il, (1, HQ))                      # [S, 512]
    sin_rep = np.tile(sin_il, (1, HQ))
    cos_t = np.ascontiguousarray(
        cos_rep.reshape(T, 128, E).transpose(1, 0, 2)).astype(bf)
    sin_t = np.ascontiguousarray(
        sin_rep.reshape(T, 128, E).transpose(1, 0, 2)).astype(bf)

    m2 = np.asarray(mask, dtype=np.float32)
    # per sk-chunk kc: the 128-wide partially-masked column block of the
    # transposed mask, as 0/1 visibility. mtd_t[p, kc, j] for global key
    # row 128*kc+p, query col 512*cq + 128*(kc-4*cq) + j.
    mtd_t = np.zeros((128, KC, 128), dtype=np.float32)
    for kc in range(KC):
        cq = kc // 4
        j0 = 128 * (kc - 4 * cq)
        qlo = 512 * cq + j0
        blk = m2[qlo:qlo + 128, 128 * kc:128 * (kc + 1)].T  # [sk 128, sq 128]
        mtd_t[:, kc, :] = np.where(blk > -1e29, 0.0, -3000.0)
    mtd_t = np.ascontiguousarray(mtd_t).astype(bf)

    wqf = np.asarray(wq, dtype=np.float32)
    wkf = np.asarray(wk, dtype=np.float32)
    wvf = np.asarray(wv, dtype=np.float32)
    wof = np.asarray(wo, dtype=np.float32)

    in_maps = []
    for c in range(N_CORES):
        wq_c = wqf[E * c:E * (c + 1), :]                    # [512, D]
        wqt = np.ascontiguousarray(
            wq_c.T.reshape(DC, 128, E).transpose(1, 0, 2)).astype(bf)
        wk_c = wkf[HD * c:HD * (c + 1), :]                  # [128, D]
        wv_c = wvf[HD * c:HD * (c + 1), :]
        wkv_c = np.concatenate([wk_c, wv_c], axis=0)        # [256, D]
        wkvt = np.ascontiguousarray(
            wkv_c.T.reshape(DC, 128, 2 * HD).transpose(1, 0, 2)).astype(bf)
        wo_c = wof[:, E * c:E * (c + 1)].T                  # [512 e, D]
        wo_t = np.ascontiguousarray(
            wo_c.reshape(HQ, 128, D).transpose(1, 0, 2)).astype(bf)
        in_maps.append({
            "xtt": xtt, "wqt": wqt, "wkvt": wkvt, "wo": wo_t,
            "mtd": mtd_t, "cos": cos_t, "sin": sin_t,
        })
    return in_maps


def kernel(x, wq, wk, wv, wo, freqs_cis, mask, start_pos=0):
    if "nc" not in _CACHE:
        _CACHE["nc"] = build_bass()
    nc = _CACHE["nc"]
    in_maps = _prep_inputs(x, wq, wk, wv, wo, freqs_cis, mask)
    res = run_bass_kernel_spmd(nc, in_maps, list(range(N_CORES)))
    total = np.zeros((S, D), dtype=np.float32)
    for c in range(N_CORES):
        total += res.results[c]["out"].astype(np.float32)
    return total.reshape(1, S, D)



# revision 44
# speedup vs baseline: 1.0855x; 1.0782x over previous
"""Trainium2 Bass kernel for GQA attention prefill (nn_Attention).

Reference semantics (b=1, s=2048, dim=4096, 32 q heads, 8 kv heads, hd=128):
  xq = x @ wq.T ; xk = x @ wk.T ; xv = x @ wv.T
  xq, xk = rope(xq), rope(xk) ; xq, xk = rmsnorm(xq), rmsnorm(xk)
  o = softmax(q k^T / sqrt(hd) + mask) v          (grouped: 4 q heads / kv head)
  out = o @ wo.T

Sharding: tensor-parallel over heads on 8 cores — core c owns q heads
4c..4c+3 and kv head c; wo is sharded on its input dim; per-core partial
outputs are summed on the host.

Per-core pipeline (bf16 matmuls, fp32 accumulate):
  A: proj [s,e] -> rope+rmsnorm -> PE-transpose q,k to [hd,s].  The first
     attention block B(0) is interleaved into A's tail (its QK->exp->PV
     chains hide under A's dense projection stream).
  B: scores^T[sk,sq] = kT.T @ qT per 128-key chunk; causal mask applied as
     an additive -3000 identity-matmul into the scores psum (keeps the
     mask off the DVE critical path); exp on ACT; PV and a ones[128,128]
     denominator matmul accumulate in psum (the ones stationary broadcasts
     the denominator to all 128 rows, so normalization is just a DVE
     reciprocal+multiply, no partition_broadcast).  Heads run in pairs so
     two independent QK->exp->PV chains overlap.
  C: out[s,d] += oT.T @ wo, interleaved one pw-group at a time into the
     NEXT B block's kc loop (pw psum evacuated on ACT) so the PE never
     idles on exp latency or the normalization chains.
Causality: fully-masked blocks are skipped entirely.
Timing discipline: every cross-engine wait the PE can hit is covered by
independent queued matmul work (phase-C drains at pair boundaries).
"""

import math
import numpy as np
import ml_dtypes

import concourse.bass as bass
import concourse.tile as tile
from concourse import bacc, mybir, bass_isa
from concourse.bass import ts
from concourse.masks import make_identity
from concourse.bass_utils import run_bass_kernel_spmd

BF16 = mybir.dt.bfloat16
FP32 = mybir.dt.float32
FP8 = mybir.dt.float8e4

N_CORES = 8
S = 2048          # sequence
D = 4096          # model dim
HD = 128          # head dim
HQ = 4            # q heads per core
E = HQ * HD       # q out dim per core (512)
T = S // 128      # 16 s-tiles of 128
CQ = S // 512     # 4 sq chunks of 512
KC = S // 128     # 16 sk chunks of 128
DC = 32           # d chunks of 128
EPS = 1e-5
INV_SQRT_HD = 1.0 / math.sqrt(HD)
SQRT_HD = math.sqrt(HD)

_CACHE = {}


def _pin_act_tables():
    """Keep every ACT function in one table set so no per-tile table
    reloads are emitted (Exp/Ln/Square/Copy all live in
    natural_log_exp_and_others)."""
    import functools
    import concourse.hw_specs as hw_specs
    import concourse.bass_interp as bass_interp
    orig = hw_specs.get_activation_tables

    @functools.cache
    def patched(module_arch):
        tabs = orig(module_arch)
        keep = "natural_log_exp_and_others"
        if keep not in tabs:
            return tabs
        E = mybir.ActivationFunctionType
        mine = {f for f in (getattr(E, n, None) for n in
                            ("Exp", "Ln", "Square", "Copy", "Identity"))
                if f is not None} & tabs[keep]
        # preserve set order/indices (act_func_set_id is positional); just
        # make `keep` the only set containing the functions this kernel uses
        return {name: (fns if name == keep else fns - mine)
                for name, fns in tabs.items()}

    bacc.get_activation_tables = patched
    bass_interp.get_activation_tables = patched


def build_bass(pin_tables=True, repeat=1, phases="ABC", denom_pe=True,
               fp8_proj=False, no_exp=False, no_chain=False, no_pd=False,
               no_mask=False, b0_in_a=True, loop_drain=0, dump_alt=False,
               qk_prefetch=0):
    if pin_tables:
        _pin_act_tables()
    nc = bacc.Bacc("TRN2", target_bir_lowering=False, debug=False,
                   num_devices=N_CORES)

    xtt = nc.dram_tensor("xtt", [T, 128, DC, 128], BF16, kind="ExternalInput").ap()
    wqt = nc.dram_tensor("wqt", [128, DC, E], BF16, kind="ExternalInput").ap()
    wkvt = nc.dram_tensor("wkvt", [128, DC, 2 * HD], BF16, kind="ExternalInput").ap()
    wo = nc.dram_tensor("wo", [128, HQ, D], BF16, kind="ExternalInput").ap()
    mtd = nc.dram_tensor("mtd", [128, KC, 128], BF16, kind="ExternalInput").ap()
    cos = nc.dram_tensor("cos", [128, T, E], BF16, kind="ExternalInput").ap()
    sin = nc.dram_tensor("sin", [128, T, E], BF16, kind="ExternalInput").ap()
    out = nc.dram_tensor("out", [S, D], mybir.dt.float16, kind="ExternalOutput").ap()

    with tile.TileContext(nc) as tc:
        _emit(nc, tc, xtt, wqt, wkvt, wo, mtd, cos, sin, out, repeat=repeat,
              phases=phases, denom_pe=denom_pe, no_exp=no_exp,
              no_chain=no_chain, no_pd=no_pd, no_mask=no_mask,
              fp8_proj=fp8_proj, b0_in_a=b0_in_a, loop_drain=loop_drain,
              dump_alt=dump_alt, qk_prefetch=qk_prefetch)
    nc.compile()
    return nc


def _emit(nc, tc, xtt, wqt, wkvt, wo, mtd, cos, sin, out, repeat=1,
          phases="ABC", denom_pe=True, no_exp=False, no_chain=False,
          no_pd=False, no_mask=False, fp8_proj=False, b0_in_a=True,
          loop_drain=0, dump_alt=False, qk_prefetch=0):
    from contextlib import ExitStack
    ctx = ExitStack()
    with ctx:
        res = ctx.enter_context(tc.tile_pool(name="res", bufs=1))
        xp = ctx.enter_context(tc.tile_pool(name="xp", bufs=2))
        fq = ctx.enter_context(tc.tile_pool(name="fq", bufs=2))
        sml = ctx.enter_context(tc.tile_pool(name="sml", bufs=2))
        pbuf = ctx.enter_context(tc.tile_pool(name="pbuf", bufs=16))
        accp = ctx.enter_context(tc.tile_pool(name="accp", bufs=3))
        stg = ctx.enter_context(tc.tile_pool(name="stg", bufs=6))
        psum = ctx.enter_context(tc.tile_pool(name="psum", bufs=1, space="PSUM"))

        # resident tensors
        if not fp8_proj:
            wq_sb = res.tile([128, DC, E], BF16)
            wkv_sb = res.tile([128, DC, 2 * HD], BF16)
        wo_sb = res.tile([128, HQ, D], BF16)
        mtd_sb = res.tile([128, KC, 128], BF16)

        vsb = res.tile([128, T, HD], BF16)       # v, [s, hd] layout
        qkT = res.tile([128, 5, T, 128], BF16)   # slots 0-3: qT heads, 4: kT
        oT = res.tile([128, HQ, T, 128], BF16)   # o^T per head: [hd, s]

        ident = res.tile([128, 128], BF16)
        make_identity(nc, ident[:])
        epsb = res.tile([128, 1], FP32)
        nc.vector.memset(epsb[:], EPS)
        ones = res.tile([128, 128], BF16)
        nc.vector.memset(ones[:], 1.0)
        if no_exp:
            cpt = res.tile([128, 512], BF16)
            nc.vector.memset(cpt[:], 0.01)
        if fp8_proj:
            xc8 = res.tile([128, DC, 128], FP8)
            nc.vector.memset(xc8[:], 0.0)
            wq8 = res.tile([128, DC, E], FP8)
            nc.vector.memset(wq8[:], 0.0)
            wkv8 = res.tile([128, DC, 2 * HD], FP8)
            nc.vector.memset(wkv8[:], 0.0)

        AF = mybir.ActivationFunctionType
        MUL = mybir.AluOpType.mult
        ADD = mybir.AluOpType.add

        # psum budget (8 banks): mix 4 {psq,pskv in A; ps in B} +
        # aux 2 {ptr in A; pw in B/C-interleave} + po 2
        PA = dict(tag="mix", bufs=4)
        AUX = dict(tag="aux", bufs=2)
        PS = dict(tag="mix", bufs=4)
        PO = dict(tag="po", bufs=2)

        loop_ctx = tc.For_i(0, repeat, 1) if repeat > 1 else None
        if loop_ctx is not None:
            ctx.enter_context(loop_ctx)

        # ---------------- Phase A: proj + rope + rmsnorm + transposes
        # Software-pipelined: PE stream is [proj(m), transposes(m-1), ...] so
        # the DVE/ACT chain of tile m runs under proj(m+1)'s matmuls.
        def a_proj(m, first):
            xc = xp.tile([128, DC, 128], BF16, tag="xc", name=f"xc{m}")
            if m == 0:
                # first accumulation step's operands lead the DMA queue
                nc.sync.dma_start(xc[:, 0:8, :], xtt[m][:, 0:8, :])
                if not fp8_proj:
                    nc.sync.dma_start(wq_sb[:, 0:8, :], wqt[:, 0:8, :])
                for g in range(8, DC, 8):
                    nc.sync.dma_start(xc[:, g:g + 8, :], xtt[m][:, g:g + 8, :])
                    if not fp8_proj:
                        nc.sync.dma_start(wq_sb[:, g:g + 8, :],
                                          wqt[:, g:g + 8, :])
                if not fp8_proj:
                    for g in range(0, DC, 8):
                        nc.sync.dma_start(wkv_sb[:, g:g + 8, :],
                                          wkvt[:, g:g + 8, :])
            elif m < 2:
                for g in range(0, DC, 8):
                    nc.sync.dma_start(xc[:, g:g + 8, :], xtt[m][:, g:g + 8, :])
            else:
                nc.sync.dma_start(xc[:], xtt[m])
            cs = fq.tile([128, E], BF16, tag="cos", name=f"cs{m}")
            nc.sync.dma_start(cs[:], cos[:, m, :])
            sn = fq.tile([128, E], BF16, tag="sin", name=f"sn{m}")
            nc.sync.dma_start(sn[:], sin[:, m, :])
            if m == 0:
                nc.sync.dma_start(mtd_sb[:], mtd[:])
            if m == 2:
                nc.sync.dma_start(wo_sb[:], wo[:])

            psq = psum.tile([128, E], FP32, name=f"psq{m}", **PA)
            pskv = psum.tile([128, 2 * HD], FP32, name=f"pskv{m}", **PA)
            if fp8_proj:
                DR = mybir.MatmulPerfMode.DoubleRow
                # fp8_proj==2 mimics the 3-term residual split (3 passes)
                passes = 3 if fp8_proj == 2 else 1
                NP = DC // 2
                for p in range(passes):
                    for t in range(NP):
                        nc.tensor.matmul(psq[:], xc8[:, 2 * t:2 * t + 2, :],
                                         wq8[:, 2 * t:2 * t + 2, :],
                                         start=(p == 0 and t == 0),
                                         stop=(p == passes - 1 and t == NP - 1),
                                         perf_mode=DR)
                for p in range(passes):
                    for t in range(NP):
                        nc.tensor.matmul(pskv[:], xc8[:, 2 * t:2 * t + 2, :],
                                         wkv8[:, 2 * t:2 * t + 2, :],
                                         start=(p == 0 and t == 0),
                                         stop=(p == passes - 1 and t == NP - 1),
                                         perf_mode=DR)
            else:
                for c in range(DC):
                    nc.tensor.matmul(psq[:], xc[:, c, :], wq_sb[:, c, :],
                                     start=(c == 0), stop=(c == DC - 1))
                for c in range(DC):
                    nc.tensor.matmul(pskv[:], xc[:, c, :], wkv_sb[:, c, :],
                                     start=(c == 0), stop=(c == DC - 1))

            # evacuate psum to sbuf bf16 (ACT), then rope on DVE in 2x mode
            qsb = sml.tile([128, E], BF16, tag="qsb", name=f"qsb{m}")
            nc.scalar.copy(qsb[:], psq[:])
            kvsb = sml.tile([128, 2 * HD], BF16, tag="kvsb", name=f"kvsb{m}")
            nc.scalar.copy(kvsb[:], pskv[:])
            nc.vector.tensor_copy(vsb[:, m, :], kvsb[:, HD:2 * HD])

            # rope(q): qro = q*cos + swap(q)*sin_signed
            tco = sml.tile([128, E], BF16, tag="tco", name=f"tco{m}")
            nc.vector.tensor_tensor(tco[:], qsb[:], cs[:], op=MUL)
            tro = sml.tile([128, E], BF16, tag="tro", name=f"tro{m}")
            q3 = qsb[:].rearrange("p (x two) -> p x two", two=2)
            t3 = tro[:].rearrange("p (x two) -> p x two", two=2)
            nc.vector.tensor_copy(t3[:, :, 0], q3[:, :, 1])
            nc.vector.tensor_copy(t3[:, :, 1], q3[:, :, 0])
            qro = sml.tile([128, E], BF16, tag="qro", name=f"qro{m}")
            nc.vector.tensor_tensor(tro[:], tro[:], sn[:], op=MUL)
            nc.vector.tensor_tensor(qro[:], tco[:], tro[:], op=ADD)

            # rope(k)
            tck = sml.tile([128, E], BF16, tag="tco", name=f"tck{m}")
            nc.vector.tensor_tensor(tck[:, 0:HD], kvsb[:, 0:HD], cs[:, 0:HD], op=MUL)
            trk = sml.tile([128, E], BF16, tag="tro", name=f"trk{m}")
            k3 = kvsb[:, 0:HD].rearrange("p (x two) -> p x two", two=2)
            r3 = trk[:, 0:HD].rearrange("p (x two) -> p x two", two=2)
            nc.vector.tensor_copy(r3[:, :, 0], k3[:, :, 1])
            nc.vector.tensor_copy(r3[:, :, 1], k3[:, :, 0])
            kro = sml.tile([128, E], BF16, tag="qro", name=f"kro{m}")
            nc.vector.tensor_tensor(trk[:, 0:HD], trk[:, 0:HD], sn[:, 0:HD], op=MUL)
            nc.vector.tensor_tensor(kro[:, 0:HD], tck[:, 0:HD], trk[:, 0:HD], op=ADD)

            # rmsnorm: rinv = exp(-0.5*ln(mean(t^2)+eps)), all in one ACT set
            sqs = sml.tile([128, 5], FP32, tag="sqs", name=f"sqs{m}")
            scr = sml.tile([128, HD], FP32, tag="scr", name=f"scr{m}")
            for h in range(HQ):
                nc.scalar.activation(scr[:], qro[:, ts(h, HD)], AF.Square,
                                     accum_out=sqs[:, h:h + 1])
            nc.scalar.activation(scr[:], kro[:, 0:HD], AF.Square,
                                 accum_out=sqs[:, 4:5])
            rin = sml.tile([128, 5], FP32, tag="rin", name=f"rin{m}")
            nc.scalar.activation(rin[:], sqs[:], AF.Ln, scale=1.0 / HD, bias=epsb[:])
            nc.scalar.activation(rin[:], rin[:], AF.Exp, scale=-0.5)

            qnt = sml.tile([128, E], BF16, tag="qnt", bufs=3, name=f"qnt{m}")
            knt = sml.tile([128, HD], BF16, tag="knt", bufs=3, name=f"knt{m}")
            for h in range(HQ):
                nc.scalar.mul(qnt[:, ts(h, HD)], qro[:, ts(h, HD)], rin[:, h:h + 1])
            nc.scalar.mul(knt[:], kro[:, 0:HD], rin[:, 4:5])
            return qnt, knt

        def a_post(m, qnt, knt):
            # transposes packed into one psum bank, single evac
            ptr = psum.tile([128, 5, 128], BF16, name=f"ptr{m}", **AUX)
            for h in range(HQ):
                nc.tensor.transpose(ptr[:, h, :], qnt[:, ts(h, HD)], ident[:])
            nc.tensor.transpose(ptr[:, 4, :], knt[:], ident[:])
            nc.scalar.copy(qkT[:, :, m, :], ptr[:])

        # ---- B(0) emitted as closures interleaved into phase A's tail:
        # its QK->exp->PV chains hide completely under A's dense projection
        # stream (deps: qkT/vsb for kc 0-3 are ready after a_post(3), i.e.
        # from m-slot 5 on).
        b0_in_a = (b0_in_a and "B" in phases and denom_pe
                   and not (no_exp or no_chain or no_pd))
        b0q = []
        if b0_in_a:
            b0state = {}

            def b0_qk(h, kc):
                j0 = 128 * kc
                ps = psum.tile([128, 512], FP32, name=f"b0sc{h}_{kc}", **PS)
                nc.tensor.matmul(ps[:, j0:], qkT[:, 4, kc, :],
                                 qkT[:, h, kc:4, :],
                                 start=True, stop=False,
                                 skip_group_check=True)
                j1 = min(j0 + 128, 512)
                nc.tensor.matmul(ps[:, j0:j1], ident[:], mtd_sb[:, kc, :],
                                 start=False, stop=True,
                                 skip_group_check=True)
                pt = pbuf.tile([128, 512], BF16, tag="pt",
                               name=f"b0pt{h}_{kc}")
                nc.scalar.activation(pt[:, j0:], ps[:, j0:], AF.Exp,
                                     scale=INV_SQRT_HD)
                b0state[(h, kc)] = pt

            def b0_pv(h, kc):
                j0 = 128 * kc
                pt = b0state.pop((h, kc))
                nc.tensor.matmul(b0state[("po", h)][:, j0:], vsb[:, kc, :],
                                 pt[:, j0:], start=(kc == 0), stop=(kc == 3))
                nc.tensor.matmul(b0state[("pd", h)][:, j0:], ones[:],
                                 pt[:, j0:], start=(kc == 0), stop=(kc == 3))

            def mk_b0_steps(h):
                def s0():
                    b0state[("po", h)] = psum.tile(
                        [128, 512], FP32, name=f"b0po{h}", **PO)
                    b0state[("pd", h)] = psum.tile(
                        [128, 512], FP32, name=f"b0pd{h}", **PO)
                    b0_qk(h, 0)
                    b0_qk(h, 1)
                    b0_pv(h, 0)

                def s1():
                    b0_qk(h, 2)
                    b0_pv(h, 1)

                def s2():
                    b0_qk(h, 3)
                    b0_pv(h, 2)

                def s3():
                    b0_pv(h, 3)

                def s4():
                    po_ = b0state.pop(("po", h))
                    pd_ = b0state.pop(("pd", h))
                    rb = accp.tile([128, 512], FP32, tag="rb",
                                   name=f"b0rb{h}")
                    nc.vector.reciprocal(rb[:], pd_[:])
                    nc.vector.tensor_tensor(oT[:, h, 0:4, :], po_[:], rb[:],
                                            op=MUL)
                return [s0, s1, s2, s3, s4]

            for h in range(HQ):
                b0q.extend(mk_b0_steps(h))

        prev = None
        for m in range(T):
            qk = a_proj(m, first=(m == 0))
            if prev is not None:
                a_post(m - 1, *prev)
            prev = qk
            if m >= 5:
                for _ in range(2):
                    if b0q:
                        b0q.pop(0)()
        a_post(T - 1, *prev)
        while b0q:
            b0q.pop(0)()

        if "B" not in phases:
            so = stg.tile([128, 512], mybir.dt.float16, tag="so", name="dummy")
            nc.vector.tensor_copy(so[:], qkT[:, 0, 0:4, :])
            nc.sync.dma_start(out[0:128, 0:512], so[:])
            return

        # ---------------- Phase B + interleaved C.
        # Heads run in PAIRS (two independent QK->exp->PV chains hide the
        # ACT exp latency from the PE), and phase C of block cq-1 is
        # interleaved one pw-group at a time into B(cq)'s kc loop so the PE
        # never idles waiting on ACT.
        def c_group(m, dc, alt=False):
            pw = psum.tile([128, 512], FP32, name=f"pw{m}_{dc}", **AUX)
            for j in range(HQ):
                nc.tensor.matmul(pw[:], oT[:, j, m, :],
                                 wo_sb[:, j, ts(dc, 512)],
                                 start=(j == 0), stop=(j == HQ - 1))
            so = stg.tile([128, 512], mybir.dt.float16, tag="so",
                          name=f"so{m}_{dc}")
            # psum evacuation on ACT by default: DVE must stay clear for
            # the normalization chains that gate the po/pd psum rings.
            # In bulk dumps (no chains in flight) alternate with DVE so
            # the aux-ring turnaround is not ACT-paced.
            if alt and dc % 2 == 1:
                nc.vector.tensor_copy(so[:], pw[:])
            else:
                nc.scalar.copy(so[:], pw[:])
            nc.sync.dma_start(out[ts(m, 128), ts(dc, 512)], so[:])

        pending = []  # deferred phase-C (m, dc) groups from the previous cq
        if b0_in_a and "C" in phases:
            pending = [(m, dc) for m in range(4) for dc in range(8)]

        def drain(n, alt=False):
            for _ in range(min(n, len(pending))):
                m, dc = pending.pop(0)
                c_group(m, dc, alt=alt)

        for cq in range(1 if b0_in_a else 0, CQ):
            nkc = 4 * cq + 4
            prefetched = None
            for hp in range(HQ // 2):
                h0, h1 = 2 * hp, 2 * hp + 1
                po0 = psum.tile([128, 512], FP32, name=f"po{cq}_{h0}", **PO)
                po1 = psum.tile([128, 512], FP32, name=f"po{cq}_{h1}", **PO)
                if denom_pe:
                    pd0 = psum.tile([128, 512], FP32, name=f"pd{cq}_{h0}",
                                    tag="mix", bufs=4)
                    pd1 = psum.tile([128, 512], FP32, name=f"pd{cq}_{h1}",
                                    tag="mix", bufs=4)
                else:
                    # denominator accumulators on DVE (quad-tree over probs
                    # tiles; fp16 intermediates, fp32 acc) keep the PE free
                    acc0 = accp.tile([128, 512], FP32, tag="acc", bufs=2,
                                     name=f"acc{cq}_{h0}")
                    acc1 = accp.tile([128, 512], FP32, tag="acc", bufs=2,
                                     name=f"acc{cq}_{h1}")

                def qk_exp(h, kc):
                    # columns below j0 are fully masked (causal): skip them
                    j0 = max(0, 128 * (kc - 4 * cq))
                    ps = psum.tile([128, 512], FP32,
                                   name=f"sc{cq}_{h}_{kc}", **PS)
                    d0 = j0 // 128
                    nc.tensor.matmul(ps[:, j0:], qkT[:, 4, kc, :],
                                     qkT[:, h, 4 * cq + d0:4 * cq + 4, :],
                                     start=True, stop=(kc < 4 * cq),
                                     skip_group_check=(kc >= 4 * cq))
                    if kc >= 4 * cq and not no_mask:
                        # diagonal block: add -3000 to masked score positions
                        # (PE identity-matmul) so exp() yields exact zeros --
                        # keeps the mask off the DVE critical path
                        j1 = min(j0 + 128, 512)
                        nc.tensor.matmul(ps[:, j0:j1], ident[:],
                                         mtd_sb[:, kc, :],
                                         start=False, stop=True,
                                         skip_group_check=True)
                    if no_exp:
                        return cpt
                    pt = pbuf.tile([128, 512], BF16, tag="pt",
                                   name=f"pt{cq}_{h}_{kc}")
                    nc.scalar.activation(pt[:, j0:], ps[:, j0:], AF.Exp,
                                         scale=INV_SQRT_HD)
                    if j0 > 0 and not denom_pe:
                        # zero fully-masked columns so the denominator tree
                        # can sum full-width tiles
                        nc.gpsimd.memset(pt[:, 0:j0], 0.0)
                    return pt

                def pv(po, pd, kc, pt):
                    j0 = max(0, 128 * (kc - 4 * cq))
                    nc.tensor.matmul(po[:, j0:], vsb[:, kc, :], pt[:, j0:],
                                     start=(kc == 0), stop=(kc == nkc - 1))
                    if denom_pe:
                        nc.tensor.matmul(pd[:, j0:], ones[:], pt[:, j0:],
                                         start=(kc == 0),
                                         stop=(kc == nkc - 1))

                FP16 = mybir.dt.float16

                def den_quad(acc, g, pts4):
                    # acc (+)= pts4[0]+pts4[1]+pts4[2]+pts4[3], on DVE
                    t1 = accp.tile([128, 512], FP16, tag="t1", bufs=2,
                                   name=f"t1_{cq}_{g}")
                    t2 = accp.tile([128, 512], FP16, tag="t2", bufs=2,
                                   name=f"t2_{cq}_{g}")
                    nc.vector.tensor_tensor(t1[:], pts4[0][:], pts4[1][:],
                                            op=ADD)
                    nc.vector.tensor_tensor(t2[:], pts4[2][:], pts4[3][:],
                                            op=ADD)
                    if g == 0:
                        nc.vector.tensor_tensor(acc[:], t1[:], t2[:], op=ADD)
                    else:
                        nc.vector.tensor_tensor(t1[:], t1[:], t2[:], op=ADD)
                        nc.vector.tensor_tensor(acc[:], acc[:], t1[:], op=ADD)

                if prefetched is not None:
                    pts0, pts1 = prefetched
                    prefetched = None
                else:
                    pts0 = [qk_exp(h0, 0)]
                    pts1 = [qk_exp(h1, 0)]
                for kc in range(nkc):
                    if kc + 1 < nkc:
                        pts0.append(qk_exp(h0, kc + 1))
                    pv(po0, pd0 if denom_pe else None, kc, pts0[kc])
                    if kc + 1 < nkc:
                        pts1.append(qk_exp(h1, kc + 1))
                    pv(po1, pd1 if denom_pe else None, kc, pts1[kc])
                    if loop_drain and kc % 2 == 1:
                        drain(loop_drain)
                    if kc % 4 == 3 and not denom_pe:
                        den_quad(acc0, kc // 4, pts0[kc - 3:kc + 1])
                        den_quad(acc1, kc // 4, pts1[kc - 3:kc + 1])

                if qk_prefetch and hp == 0 and denom_pe:
                    # issue the next pair's first QKs now: their exps
                    # complete during the drain burst below, so the next
                    # pair's first PVs start hot
                    prefetched = ([qk_exp(h0 + 2, 0)], [qk_exp(h1 + 2, 0)])
                # cover the denominator/normalization chain (and the po
                # psum-ring reuse it gates) with independent phase-C work
                drain(6)
                if not no_chain:
                    if denom_pe:
                        for h, po_, pd_ in ((h0, po0, pd0), (h1, po1, pd1)):
                            rb = accp.tile([128, 512], FP32, tag="rb",
                                           name=f"rb{cq}_{h}")
                            nc.vector.reciprocal(rb[:], pd_[:])
                            nc.vector.tensor_tensor(oT[:, h, ts(cq, 4), :],
                                                    po_[:], rb[:], op=MUL)
                    else:
                        for h, po_, acc_ in ((h0, po0, acc0), (h1, po1, acc1)):
                            dn = accp.tile([128, 512], FP32, tag="dn", bufs=2,
                                           name=f"dn{cq}_{h}")
                            nc.gpsimd.partition_all_reduce(
                                dn[:], acc_[:], channels=128,
                                reduce_op=bass_isa.ReduceOp.add)
                            rb = accp.tile([128, 512], FP32, tag="rb",
                                           name=f"rb{cq}_{h}")
                            nc.vector.reciprocal(rb[:], dn[:])
                            nc.vector.tensor_tensor(oT[:, h, ts(cq, 4), :],
                                                    po_[:], rb[:], op=MUL)

            if "C" in phases:
                # bulk dump: chains already issued, DVE free
                drain(len(pending), alt=dump_alt)
                pending = [(m, dc) for m in range(4 * cq, 4 * cq + 4)
                           for dc in range(8)]

        if "C" in phases:
            drain(len(pending), alt=dump_alt)
        elif "B" in phases:
            so = stg.tile([128, 512], mybir.dt.float16, tag="so", name="dummyC")
            src = qkT[:, 0, 0:4, :] if no_chain else oT[:, 0, 0:4, :]
            nc.vector.tensor_copy(so[:], src)
            nc.sync.dma_start(out[0:128, 0:512], so[:])


def _prep_inputs(x, wq, wk, wv, wo, freqs_cis, mask):
    """Host-side shard + retile. Returns list of per-core input dicts."""
    bf = ml_dtypes.bfloat16
    x2 = np.asarray(x, dtype=np.float32).reshape(S, D)
    # xtt[m, p, c, s'] = x[128m+s', 128c+p]
    xtt = np.ascontiguousarray(
        x2.reshape(T, 128, DC, 128).transpose(0, 3, 2, 1)).astype(bf)

    fr = np.asarray(freqs_cis, dtype=np.float32)[..., 0]   # [S, 64]
    fi = np.asarray(freqs_cis, dtype=np.float32)[..., 1]
    cos_il = np.repeat(fr, 2, axis=1)                       # [S, 128]
    sin_il = np.repeat(fi, 2, axis=1)
    sin_il[:, 0::2] *= -1.0                                 # signed for swap-form
    cos_rep = np.tile(cos_il, (1, HQ))                      # [S, 512]
    sin_rep = np.tile(sin_il, (1, HQ))
    cos_t = np.ascontiguousarray(
        cos_rep.reshape(T, 128, E).transpose(1, 0, 2)).astype(bf)
    sin_t = np.ascontiguousarray(
        sin_rep.reshape(T, 128, E).transpose(1, 0, 2)).astype(bf)

    m2 = np.asarray(mask, dtype=np.float32)
    # per sk-chunk kc: the 128-wide partially-masked column block of the
    # transposed mask, as 0/1 visibility. mtd_t[p, kc, j] for global key
    # row 128*kc+p, query col 512*cq + 128*(kc-4*cq) + j.
    mtd_t = np.zeros((128, KC, 128), dtype=np.float32)
    for kc in range(KC):
        cq = kc // 4
        j0 = 128 * (kc - 4 * cq)
        qlo = 512 * cq + j0
        blk = m2[qlo:qlo + 128, 128 * kc:128 * (kc + 1)].T  # [sk 128, sq 128]
        mtd_t[:, kc, :] = np.where(blk > -1e29, 0.0, -3000.0)
    mtd_t = np.ascontiguousarray(mtd_t).astype(bf)

    wqf = np.asarray(wq, dtype=np.float32)
    wkf = np.asarray(wk, dtype=np.float32)
    wvf = np.asarray(wv, dtype=np.float32)
    wof = np.asarray(wo, dtype=np.float32)

    in_maps = []
    for c in range(N_CORES):
        wq_c = wqf[E * c:E * (c + 1), :]                    # [512, D]
        wqt = np.ascontiguousarray(
            wq_c.T.reshape(DC, 128, E).transpose(1, 0, 2)).astype(bf)
        wk_c = wkf[HD * c:HD * (c + 1), :]                  # [128, D]
        wv_c = wvf[HD * c:HD * (c + 1), :]
        wkv_c = np.concatenate([wk_c, wv_c], axis=0)        # [256, D]
        wkvt = np.ascontiguousarray(
            wkv_c.T.reshape(DC, 128, 2 * HD).transpose(1, 0, 2)).astype(bf)
        wo_c = wof[:, E * c:E * (c + 1)].T                  # [512 e, D]
        wo_t = np.ascontiguousarray(
            wo_c.reshape(HQ, 128, D).transpose(1, 0, 2)).astype(bf)
        in_maps.append({
            "xtt": xtt, "wqt": wqt, "wkvt": wkvt, "wo": wo_t,
            "mtd": mtd_t, "cos": cos_t, "sin": sin_t,
        })
    return in_maps


def kernel(x, wq, wk, wv, wo, freqs_cis, mask, start_pos=0):
    if "nc" not in _CACHE:
        _CACHE["nc"] = build_bass()
    nc = _CACHE["nc"]
    in_maps = _prep_inputs(x, wq, wk, wv, wo, freqs_cis, mask)
    res = run_bass_kernel_spmd(nc, in_maps, list(range(N_CORES)))
    total = np.zeros((S, D), dtype=np.float32)
    for c in range(N_CORES):
        total += res.results[c]["out"].astype(np.float32)
    return total.reshape(1, S, D)



# revision 45
# speedup vs baseline: 1.1160x; 1.0281x over previous
"""Trainium2 Bass kernel for GQA attention prefill (nn_Attention).

Reference semantics (b=1, s=2048, dim=4096, 32 q heads, 8 kv heads, hd=128):
  xq = x @ wq.T ; xk = x @ wk.T ; xv = x @ wv.T
  xq, xk = rope(xq), rope(xk) ; xq, xk = rmsnorm(xq), rmsnorm(xk)
  o = softmax(q k^T / sqrt(hd) + mask) v          (grouped: 4 q heads / kv head)
  out = o @ wo.T

Sharding: tensor-parallel over heads on 8 cores — core c owns q heads
4c..4c+3 and kv head c; wo is sharded on its input dim; per-core partial
outputs are summed on the host.

Per-core pipeline (bf16 matmuls, fp32 accumulate):
  A: proj [s,e] -> rope+rmsnorm -> PE-transpose q,k to [hd,s].  The first
     attention block B(0) is interleaved into A's tail (its QK->exp->PV
     chains hide under A's dense projection stream).
  B: scores^T[sk,sq] = kT.T @ qT per 128-key chunk; causal mask applied as
     an additive -3000 identity-matmul into the scores psum (keeps the
     mask off the DVE critical path); exp on ACT; PV and a ones[128,128]
     denominator matmul accumulate in psum (the ones stationary broadcasts
     the denominator to all 128 rows, so normalization is just a DVE
     reciprocal+multiply, no partition_broadcast).  Heads run in pairs so
     two independent QK->exp->PV chains overlap.
  C: out[s,d] += oT.T @ wo, interleaved one pw-group at a time into the
     NEXT B block's kc loop (pw psum evacuated on ACT) so the PE never
     idles on exp latency or the normalization chains.
Causality: fully-masked blocks are skipped entirely.
Timing discipline: every cross-engine wait the PE can hit is covered by
independent queued matmul work (phase-C drains at pair boundaries).
"""

import math
import numpy as np
import ml_dtypes

import concourse.bass as bass
import concourse.tile as tile
from concourse import bacc, mybir, bass_isa
from concourse.bass import ts
from concourse.masks import make_identity
from concourse.bass_utils import run_bass_kernel_spmd

BF16 = mybir.dt.bfloat16
FP32 = mybir.dt.float32
FP8 = mybir.dt.float8e4

N_CORES = 8
S = 2048          # sequence
D = 4096          # model dim
HD = 128          # head dim
HQ = 4            # q heads per core
E = HQ * HD       # q out dim per core (512)
T = S // 128      # 16 s-tiles of 128
CQ = S // 512     # 4 sq chunks of 512
KC = S // 128     # 16 sk chunks of 128
DC = 32           # d chunks of 128
EPS = 1e-5
INV_SQRT_HD = 1.0 / math.sqrt(HD)
SQRT_HD = math.sqrt(HD)

_CACHE = {}


def _pin_act_tables():
    """Keep every ACT function in one table set so no per-tile table
    reloads are emitted (Exp/Ln/Square/Copy all live in
    natural_log_exp_and_others)."""
    import functools
    import concourse.hw_specs as hw_specs
    import concourse.bass_interp as bass_interp
    orig = hw_specs.get_activation_tables

    @functools.cache
    def patched(module_arch):
        tabs = orig(module_arch)
        keep = "natural_log_exp_and_others"
        if keep not in tabs:
            return tabs
        E = mybir.ActivationFunctionType
        mine = {f for f in (getattr(E, n, None) for n in
                            ("Exp", "Ln", "Square", "Copy", "Identity"))
                if f is not None} & tabs[keep]
        # preserve set order/indices (act_func_set_id is positional); just
        # make `keep` the only set containing the functions this kernel uses
        return {name: (fns if name == keep else fns - mine)
                for name, fns in tabs.items()}

    bacc.get_activation_tables = patched
    bass_interp.get_activation_tables = patched


def build_bass(pin_tables=True, repeat=1, phases="ABC", denom_pe=True,
               fp8_proj=False, no_exp=False, no_chain=False, no_pd=False,
               no_mask=False, b0_in_a=True, loop_drain=0, dump_alt=False,
               qk_prefetch=0, qk_group=0):
    if pin_tables:
        _pin_act_tables()
    nc = bacc.Bacc("TRN2", target_bir_lowering=False, debug=False,
                   num_devices=N_CORES)

    xtt = nc.dram_tensor("xtt", [T, 128, DC, 128], BF16, kind="ExternalInput").ap()
    wqt = nc.dram_tensor("wqt", [128, DC, E], BF16, kind="ExternalInput").ap()
    wkvt = nc.dram_tensor("wkvt", [128, DC, 2 * HD], BF16, kind="ExternalInput").ap()
    wo = nc.dram_tensor("wo", [128, HQ, D], BF16, kind="ExternalInput").ap()
    mtd = nc.dram_tensor("mtd", [128, KC, 128], BF16, kind="ExternalInput").ap()
    cos = nc.dram_tensor("cos", [128, T, E], BF16, kind="ExternalInput").ap()
    sin = nc.dram_tensor("sin", [128, T, E], BF16, kind="ExternalInput").ap()
    out = nc.dram_tensor("out", [S, D], mybir.dt.float16, kind="ExternalOutput").ap()

    with tile.TileContext(nc) as tc:
        _emit(nc, tc, xtt, wqt, wkvt, wo, mtd, cos, sin, out, repeat=repeat,
              phases=phases, denom_pe=denom_pe, no_exp=no_exp,
              no_chain=no_chain, no_pd=no_pd, no_mask=no_mask,
              fp8_proj=fp8_proj, b0_in_a=b0_in_a, loop_drain=loop_drain,
              dump_alt=dump_alt, qk_prefetch=qk_prefetch, qk_group=qk_group)
    nc.compile()
    return nc


def _emit(nc, tc, xtt, wqt, wkvt, wo, mtd, cos, sin, out, repeat=1,
          phases="ABC", denom_pe=True, no_exp=False, no_chain=False,
          no_pd=False, no_mask=False, fp8_proj=False, b0_in_a=True,
          loop_drain=0, dump_alt=False, qk_prefetch=0, qk_group=0):
    from contextlib import ExitStack
    ctx = ExitStack()
    with ctx:
        res = ctx.enter_context(tc.tile_pool(name="res", bufs=1))
        xp = ctx.enter_context(tc.tile_pool(name="xp", bufs=2))
        fq = ctx.enter_context(tc.tile_pool(name="fq", bufs=2))
        sml = ctx.enter_context(tc.tile_pool(name="sml", bufs=2))
        pbuf = ctx.enter_context(tc.tile_pool(name="pbuf", bufs=16))
        accp = ctx.enter_context(tc.tile_pool(name="accp", bufs=3))
        stg = ctx.enter_context(tc.tile_pool(name="stg", bufs=6))
        psum = ctx.enter_context(tc.tile_pool(name="psum", bufs=1, space="PSUM"))

        # resident tensors
        if not fp8_proj:
            wq_sb = res.tile([128, DC, E], BF16)
            wkv_sb = res.tile([128, DC, 2 * HD], BF16)
        wo_sb = res.tile([128, HQ, D], BF16)
        mtd_sb = res.tile([128, KC, 128], BF16)

        vsb = res.tile([128, T, HD], BF16)       # v, [s, hd] layout
        qkT = res.tile([128, 5, T, 128], BF16)   # slots 0-3: qT heads, 4: kT
        oT = res.tile([128, HQ, T, 128], BF16)   # o^T per head: [hd, s]

        ident = res.tile([128, 128], BF16)
        make_identity(nc, ident[:])
        epsb = res.tile([128, 1], FP32)
        nc.vector.memset(epsb[:], EPS)
        ones = res.tile([128, 128], BF16)
        nc.vector.memset(ones[:], 1.0)
        if no_exp:
            cpt = res.tile([128, 512], BF16)
            nc.vector.memset(cpt[:], 0.01)
        if fp8_proj:
            xc8 = res.tile([128, DC, 128], FP8)
            nc.vector.memset(xc8[:], 0.0)
            wq8 = res.tile([128, DC, E], FP8)
            nc.vector.memset(wq8[:], 0.0)
            wkv8 = res.tile([128, DC, 2 * HD], FP8)
            nc.vector.memset(wkv8[:], 0.0)

        AF = mybir.ActivationFunctionType
        MUL = mybir.AluOpType.mult
        ADD = mybir.AluOpType.add

        # psum budget (8 banks): mix 4 {psq,pskv in A; ps in B} +
        # aux 2 {ptr in A; pw in B/C-interleave} + po 2
        PA = dict(tag="mix", bufs=4)
        AUX = dict(tag="aux", bufs=2)
        PS = dict(tag="mix", bufs=4)
        PO = dict(tag="po", bufs=2)

        loop_ctx = tc.For_i(0, repeat, 1) if repeat > 1 else None
        if loop_ctx is not None:
            ctx.enter_context(loop_ctx)

        # ---------------- Phase A: proj + rope + rmsnorm + transposes
        # Software-pipelined: PE stream is [proj(m), transposes(m-1), ...] so
        # the DVE/ACT chain of tile m runs under proj(m+1)'s matmuls.
        def a_proj(m, first):
            xc = xp.tile([128, DC, 128], BF16, tag="xc", name=f"xc{m}")
            if m == 0:
                # first accumulation step's operands lead the DMA queue
                nc.sync.dma_start(xc[:, 0:8, :], xtt[m][:, 0:8, :])
                if not fp8_proj:
                    nc.sync.dma_start(wq_sb[:, 0:8, :], wqt[:, 0:8, :])
                for g in range(8, DC, 8):
                    nc.sync.dma_start(xc[:, g:g + 8, :], xtt[m][:, g:g + 8, :])
                    if not fp8_proj:
                        nc.sync.dma_start(wq_sb[:, g:g + 8, :],
                                          wqt[:, g:g + 8, :])
                if not fp8_proj:
                    for g in range(0, DC, 8):
                        nc.sync.dma_start(wkv_sb[:, g:g + 8, :],
                                          wkvt[:, g:g + 8, :])
            elif m < 2:
                for g in range(0, DC, 8):
                    nc.sync.dma_start(xc[:, g:g + 8, :], xtt[m][:, g:g + 8, :])
            else:
                nc.sync.dma_start(xc[:], xtt[m])
            cs = fq.tile([128, E], BF16, tag="cos", name=f"cs{m}")
            nc.sync.dma_start(cs[:], cos[:, m, :])
            sn = fq.tile([128, E], BF16, tag="sin", name=f"sn{m}")
            nc.sync.dma_start(sn[:], sin[:, m, :])
            if m == 0:
                nc.sync.dma_start(mtd_sb[:], mtd[:])
            if m == 2:
                nc.sync.dma_start(wo_sb[:], wo[:])

            psq = psum.tile([128, E], FP32, name=f"psq{m}", **PA)
            pskv = psum.tile([128, 2 * HD], FP32, name=f"pskv{m}", **PA)
            if fp8_proj:
                DR = mybir.MatmulPerfMode.DoubleRow
                # fp8_proj==2 mimics the 3-term residual split (3 passes)
                passes = 3 if fp8_proj == 2 else 1
                NP = DC // 2
                for p in range(passes):
                    for t in range(NP):
                        nc.tensor.matmul(psq[:], xc8[:, 2 * t:2 * t + 2, :],
                                         wq8[:, 2 * t:2 * t + 2, :],
                                         start=(p == 0 and t == 0),
                                         stop=(p == passes - 1 and t == NP - 1),
                                         perf_mode=DR)
                for p in range(passes):
                    for t in range(NP):
                        nc.tensor.matmul(pskv[:], xc8[:, 2 * t:2 * t + 2, :],
                                         wkv8[:, 2 * t:2 * t + 2, :],
                                         start=(p == 0 and t == 0),
                                         stop=(p == passes - 1 and t == NP - 1),
                                         perf_mode=DR)
            else:
                for c in range(DC):
                    nc.tensor.matmul(psq[:], xc[:, c, :], wq_sb[:, c, :],
                                     start=(c == 0), stop=(c == DC - 1))
                for c in range(DC):
                    nc.tensor.matmul(pskv[:], xc[:, c, :], wkv_sb[:, c, :],
                                     start=(c == 0), stop=(c == DC - 1))

            # evacuate psum to sbuf bf16 (ACT), then rope on DVE in 2x mode
            qsb = sml.tile([128, E], BF16, tag="qsb", name=f"qsb{m}")
            nc.scalar.copy(qsb[:], psq[:])
            kvsb = sml.tile([128, 2 * HD], BF16, tag="kvsb", name=f"kvsb{m}")
            nc.scalar.copy(kvsb[:], pskv[:])
            nc.vector.tensor_copy(vsb[:, m, :], kvsb[:, HD:2 * HD])

            # rope(q): qro = q*cos + swap(q)*sin_signed
            tco = sml.tile([128, E], BF16, tag="tco", name=f"tco{m}")
            nc.vector.tensor_tensor(tco[:], qsb[:], cs[:], op=MUL)
            tro = sml.tile([128, E], BF16, tag="tro", name=f"tro{m}")
            q3 = qsb[:].rearrange("p (x two) -> p x two", two=2)
            t3 = tro[:].rearrange("p (x two) -> p x two", two=2)
            nc.vector.tensor_copy(t3[:, :, 0], q3[:, :, 1])
            nc.vector.tensor_copy(t3[:, :, 1], q3[:, :, 0])
            qro = sml.tile([128, E], BF16, tag="qro", name=f"qro{m}")
            nc.vector.tensor_tensor(tro[:], tro[:], sn[:], op=MUL)
            nc.vector.tensor_tensor(qro[:], tco[:], tro[:], op=ADD)

            # rope(k)
            tck = sml.tile([128, E], BF16, tag="tco", name=f"tck{m}")
            nc.vector.tensor_tensor(tck[:, 0:HD], kvsb[:, 0:HD], cs[:, 0:HD], op=MUL)
            trk = sml.tile([128, E], BF16, tag="tro", name=f"trk{m}")
            k3 = kvsb[:, 0:HD].rearrange("p (x two) -> p x two", two=2)
            r3 = trk[:, 0:HD].rearrange("p (x two) -> p x two", two=2)
            nc.vector.tensor_copy(r3[:, :, 0], k3[:, :, 1])
            nc.vector.tensor_copy(r3[:, :, 1], k3[:, :, 0])
            kro = sml.tile([128, E], BF16, tag="qro", name=f"kro{m}")
            nc.vector.tensor_tensor(trk[:, 0:HD], trk[:, 0:HD], sn[:, 0:HD], op=MUL)
            nc.vector.tensor_tensor(kro[:, 0:HD], tck[:, 0:HD], trk[:, 0:HD], op=ADD)

            # rmsnorm: rinv = exp(-0.5*ln(mean(t^2)+eps)), all in one ACT set
            sqs = sml.tile([128, 5], FP32, tag="sqs", name=f"sqs{m}")
            scr = sml.tile([128, HD], FP32, tag="scr", name=f"scr{m}")
            for h in range(HQ):
                nc.scalar.activation(scr[:], qro[:, ts(h, HD)], AF.Square,
                                     accum_out=sqs[:, h:h + 1])
            nc.scalar.activation(scr[:], kro[:, 0:HD], AF.Square,
                                 accum_out=sqs[:, 4:5])
            rin = sml.tile([128, 5], FP32, tag="rin", name=f"rin{m}")
            nc.scalar.activation(rin[:], sqs[:], AF.Ln, scale=1.0 / HD, bias=epsb[:])
            nc.scalar.activation(rin[:], rin[:], AF.Exp, scale=-0.5)

            qnt = sml.tile([128, E], BF16, tag="qnt", bufs=3, name=f"qnt{m}")
            knt = sml.tile([128, HD], BF16, tag="knt", bufs=3, name=f"knt{m}")
            for h in range(HQ):
                nc.scalar.mul(qnt[:, ts(h, HD)], qro[:, ts(h, HD)], rin[:, h:h + 1])
            nc.scalar.mul(knt[:], kro[:, 0:HD], rin[:, 4:5])
            return qnt, knt

        def a_post(m, qnt, knt):
            # transposes packed into one psum bank, single evac
            ptr = psum.tile([128, 5, 128], BF16, name=f"ptr{m}", **AUX)
            for h in range(HQ):
                nc.tensor.transpose(ptr[:, h, :], qnt[:, ts(h, HD)], ident[:])
            nc.tensor.transpose(ptr[:, 4, :], knt[:], ident[:])
            nc.scalar.copy(qkT[:, :, m, :], ptr[:])

        # ---- B(0) emitted as closures interleaved into phase A's tail:
        # its QK->exp->PV chains hide completely under A's dense projection
        # stream (deps: qkT/vsb for kc 0-3 are ready after a_post(3), i.e.
        # from m-slot 5 on).
        b0_in_a = (b0_in_a and "B" in phases and denom_pe
                   and not (no_exp or no_chain or no_pd))
        b0q = []
        if b0_in_a:
            b0state = {}

            def b0_qk(h, kc):
                j0 = 128 * kc
                ps = psum.tile([128, 512], FP32, name=f"b0sc{h}_{kc}", **PS)
                nc.tensor.matmul(ps[:, j0:], qkT[:, 4, kc, :],
                                 qkT[:, h, kc:4, :],
                                 start=True, stop=False,
                                 skip_group_check=True)
                j1 = min(j0 + 128, 512)
                nc.tensor.matmul(ps[:, j0:j1], ident[:], mtd_sb[:, kc, :],
                                 start=False, stop=True,
                                 skip_group_check=True)
                pt = pbuf.tile([128, 512], BF16, tag="pt",
                               name=f"b0pt{h}_{kc}")
                nc.scalar.activation(pt[:, j0:], ps[:, j0:], AF.Exp,
                                     scale=INV_SQRT_HD)
                b0state[(h, kc)] = pt

            def b0_pv(h, kc):
                j0 = 128 * kc
                pt = b0state.pop((h, kc))
                nc.tensor.matmul(b0state[("po", h)][:, j0:], vsb[:, kc, :],
                                 pt[:, j0:], start=(kc == 0), stop=(kc == 3))
                nc.tensor.matmul(b0state[("pd", h)][:, j0:], ones[:],
                                 pt[:, j0:], start=(kc == 0), stop=(kc == 3))

            def mk_b0_steps(h):
                def s0():
                    b0state[("po", h)] = psum.tile(
                        [128, 512], FP32, name=f"b0po{h}", **PO)
                    b0state[("pd", h)] = psum.tile(
                        [128, 512], FP32, name=f"b0pd{h}", **PO)
                    b0_qk(h, 0)
                    b0_qk(h, 1)
                    b0_pv(h, 0)

                def s1():
                    b0_qk(h, 2)
                    b0_pv(h, 1)

                def s2():
                    b0_qk(h, 3)
                    b0_pv(h, 2)

                def s3():
                    b0_pv(h, 3)

                def s4():
                    po_ = b0state.pop(("po", h))
                    pd_ = b0state.pop(("pd", h))
                    rb = accp.tile([128, 512], FP32, tag="rb",
                                   name=f"b0rb{h}")
                    nc.vector.reciprocal(rb[:], pd_[:])
                    nc.vector.tensor_tensor(oT[:, h, 0:4, :], po_[:], rb[:],
                                            op=MUL)
                return [s0, s1, s2, s3, s4]

            for h in range(HQ):
                b0q.extend(mk_b0_steps(h))

        prev = None
        for m in range(T):
            qk = a_proj(m, first=(m == 0))
            if prev is not None:
                a_post(m - 1, *prev)
            prev = qk
            if m >= 5:
                for _ in range(2):
                    if b0q:
                        b0q.pop(0)()
        a_post(T - 1, *prev)
        while b0q:
            b0q.pop(0)()

        if "B" not in phases:
            so = stg.tile([128, 512], mybir.dt.float16, tag="so", name="dummy")
            nc.vector.tensor_copy(so[:], qkT[:, 0, 0:4, :])
            nc.sync.dma_start(out[0:128, 0:512], so[:])
            return

        # ---------------- Phase B + interleaved C.
        # Heads run in PAIRS (two independent QK->exp->PV chains hide the
        # ACT exp latency from the PE), and phase C of block cq-1 is
        # interleaved one pw-group at a time into B(cq)'s kc loop so the PE
        # never idles waiting on ACT.
        def c_group(m, dc, alt=False):
            pw = psum.tile([128, 512], FP32, name=f"pw{m}_{dc}", **AUX)
            for j in range(HQ):
                nc.tensor.matmul(pw[:], oT[:, j, m, :],
                                 wo_sb[:, j, ts(dc, 512)],
                                 start=(j == 0), stop=(j == HQ - 1))
            so = stg.tile([128, 512], mybir.dt.float16, tag="so",
                          name=f"so{m}_{dc}")
            # psum evacuation on ACT by default: DVE must stay clear for
            # the normalization chains that gate the po/pd psum rings.
            # In bulk dumps (no chains in flight) alternate with DVE so
            # the aux-ring turnaround is not ACT-paced.
            if alt and dc % 2 == 1:
                nc.vector.tensor_copy(so[:], pw[:])
            else:
                nc.scalar.copy(so[:], pw[:])
            nc.sync.dma_start(out[ts(m, 128), ts(dc, 512)], so[:])

        pending = []  # deferred phase-C (m, dc) groups from the previous cq
        if b0_in_a and "C" in phases:
            pending = [(m, dc) for m in range(4) for dc in range(8)]

        def drain(n, alt=False):
            for _ in range(min(n, len(pending))):
                m, dc = pending.pop(0)
                c_group(m, dc, alt=alt)

        for cq in range(1 if b0_in_a else 0, CQ):
            nkc = 4 * cq + 4
            prefetched = None
            for hp in range(HQ // 2):
                h0, h1 = 2 * hp, 2 * hp + 1
                po0 = psum.tile([128, 512], FP32, name=f"po{cq}_{h0}", **PO)
                po1 = psum.tile([128, 512], FP32, name=f"po{cq}_{h1}", **PO)
                if denom_pe:
                    pd0 = psum.tile([128, 512], FP32, name=f"pd{cq}_{h0}",
                                    tag="mix", bufs=4)
                    pd1 = psum.tile([128, 512], FP32, name=f"pd{cq}_{h1}",
                                    tag="mix", bufs=4)
                else:
                    # denominator accumulators on DVE (quad-tree over probs
                    # tiles; fp16 intermediates, fp32 acc) keep the PE free
                    acc0 = accp.tile([128, 512], FP32, tag="acc", bufs=2,
                                     name=f"acc{cq}_{h0}")
                    acc1 = accp.tile([128, 512], FP32, tag="acc", bufs=2,
                                     name=f"acc{cq}_{h1}")

                def qk_exp(h, kc):
                    # columns below j0 are fully masked (causal): skip them
                    j0 = max(0, 128 * (kc - 4 * cq))
                    ps = psum.tile([128, 512], FP32,
                                   name=f"sc{cq}_{h}_{kc}", **PS)
                    d0 = j0 // 128
                    nc.tensor.matmul(ps[:, j0:], qkT[:, 4, kc, :],
                                     qkT[:, h, 4 * cq + d0:4 * cq + 4, :],
                                     start=True, stop=(kc < 4 * cq),
                                     skip_group_check=(kc >= 4 * cq))
                    if kc >= 4 * cq and not no_mask:
                        # diagonal block: add -3000 to masked score positions
                        # (PE identity-matmul) so exp() yields exact zeros --
                        # keeps the mask off the DVE critical path
                        j1 = min(j0 + 128, 512)
                        nc.tensor.matmul(ps[:, j0:j1], ident[:],
                                         mtd_sb[:, kc, :],
                                         start=False, stop=True,
                                         skip_group_check=True)
                    if no_exp:
                        return cpt
                    pt = pbuf.tile([128, 512], BF16, tag="pt",
                                   name=f"pt{cq}_{h}_{kc}")
                    nc.scalar.activation(pt[:, j0:], ps[:, j0:], AF.Exp,
                                         scale=INV_SQRT_HD)
                    if j0 > 0 and not denom_pe:
                        # zero fully-masked columns so the denominator tree
                        # can sum full-width tiles
                        nc.gpsimd.memset(pt[:, 0:j0], 0.0)
                    return pt

                def pv(po, pd, kc, pt):
                    j0 = max(0, 128 * (kc - 4 * cq))
                    nc.tensor.matmul(po[:, j0:], vsb[:, kc, :], pt[:, j0:],
                                     start=(kc == 0), stop=(kc == nkc - 1))
                    if denom_pe:
                        nc.tensor.matmul(pd[:, j0:], ones[:], pt[:, j0:],
                                         start=(kc == 0),
                                         stop=(kc == nkc - 1))

                FP16 = mybir.dt.float16

                def den_quad(acc, g, pts4):
                    # acc (+)= pts4[0]+pts4[1]+pts4[2]+pts4[3], on DVE
                    t1 = accp.tile([128, 512], FP16, tag="t1", bufs=2,
                                   name=f"t1_{cq}_{g}")
                    t2 = accp.tile([128, 512], FP16, tag="t2", bufs=2,
                                   name=f"t2_{cq}_{g}")
                    nc.vector.tensor_tensor(t1[:], pts4[0][:], pts4[1][:],
                                            op=ADD)
                    nc.vector.tensor_tensor(t2[:], pts4[2][:], pts4[3][:],
                                            op=ADD)
                    if g == 0:
                        nc.vector.tensor_tensor(acc[:], t1[:], t2[:], op=ADD)
                    else:
                        nc.vector.tensor_tensor(t1[:], t1[:], t2[:], op=ADD)
                        nc.vector.tensor_tensor(acc[:], acc[:], t1[:], op=ADD)

                if prefetched is not None:
                    pts0, pts1 = prefetched
                    prefetched = None
                else:
                    pts0 = [qk_exp(h0, 0)]
                    pts1 = [qk_exp(h1, 0)]
                for kc in range(nkc):
                    if qk_group:
                        # both QKs first: their exps launch back-to-back on
                        # ACT before either PV is reached
                        if kc + 1 < nkc:
                            pts0.append(qk_exp(h0, kc + 1))
                            pts1.append(qk_exp(h1, kc + 1))
                        pv(po0, pd0 if denom_pe else None, kc, pts0[kc])
                        pv(po1, pd1 if denom_pe else None, kc, pts1[kc])
                    else:
                        if kc + 1 < nkc:
                            pts0.append(qk_exp(h0, kc + 1))
                        pv(po0, pd0 if denom_pe else None, kc, pts0[kc])
                        if kc + 1 < nkc:
                            pts1.append(qk_exp(h1, kc + 1))
                        pv(po1, pd1 if denom_pe else None, kc, pts1[kc])
                    if loop_drain and kc % 2 == 1:
                        drain(loop_drain)
                    if kc % 4 == 3 and not denom_pe:
                        den_quad(acc0, kc // 4, pts0[kc - 3:kc + 1])
                        den_quad(acc1, kc // 4, pts1[kc - 3:kc + 1])

                if qk_prefetch and hp == 0 and denom_pe:
                    # issue the next pair's first QKs now: their exps
                    # complete during the drain burst below, so the next
                    # pair's first PVs start hot
                    prefetched = ([qk_exp(h0 + 2, 0)], [qk_exp(h1 + 2, 0)])
                # cover the denominator/normalization chain (and the po
                # psum-ring reuse it gates) with independent phase-C work
                drain(6)
                if not no_chain:
                    if denom_pe:
                        for h, po_, pd_ in ((h0, po0, pd0), (h1, po1, pd1)):
                            rb = accp.tile([128, 512], FP32, tag="rb",
                                           name=f"rb{cq}_{h}")
                            nc.vector.reciprocal(rb[:], pd_[:])
                            nc.vector.tensor_tensor(oT[:, h, ts(cq, 4), :],
                                                    po_[:], rb[:], op=MUL)
                    else:
                        for h, po_, acc_ in ((h0, po0, acc0), (h1, po1, acc1)):
                            dn = accp.tile([128, 512], FP32, tag="dn", bufs=2,
                                           name=f"dn{cq}_{h}")
                            nc.gpsimd.partition_all_reduce(
                                dn[:], acc_[:], channels=128,
                                reduce_op=bass_isa.ReduceOp.add)
                            rb = accp.tile([128, 512], FP32, tag="rb",
                                           name=f"rb{cq}_{h}")
                            nc.vector.reciprocal(rb[:], dn[:])
                            nc.vector.tensor_tensor(oT[:, h, ts(cq, 4), :],
                                                    po_[:], rb[:], op=MUL)

            if "C" in phases:
                # bulk dump: chains already issued, DVE free
                drain(len(pending), alt=dump_alt)
                pending = [(m, dc) for m in range(4 * cq, 4 * cq + 4)
                           for dc in range(8)]

        if "C" in phases:
            drain(len(pending), alt=dump_alt)
        elif "B" in phases:
            so = stg.tile([128, 512], mybir.dt.float16, tag="so", name="dummyC")
            src = qkT[:, 0, 0:4, :] if no_chain else oT[:, 0, 0:4, :]
            nc.vector.tensor_copy(so[:], src)
            nc.sync.dma_start(out[0:128, 0:512], so[:])


def _prep_inputs(x, wq, wk, wv, wo, freqs_cis, mask):
    """Host-side shard + retile. Returns list of per-core input dicts."""
    bf = ml_dtypes.bfloat16
    x2 = np.asarray(x, dtype=np.float32).reshape(S, D)
    # xtt[m, p, c, s'] = x[128m+s', 128c+p]
    xtt = np.ascontiguousarray(
        x2.reshape(T, 128, DC, 128).transpose(0, 3, 2, 1)).astype(bf)

    fr = np.asarray(freqs_cis, dtype=np.float32)[..., 0]   # [S, 64]
    fi = np.asarray(freqs_cis, dtype=np.float32)[..., 1]
    cos_il = np.repeat(fr, 2, axis=1)                       # [S, 128]
    sin_il = np.repeat(fi, 2, axis=1)
    sin_il[:, 0::2] *= -1.0                                 # signed for swap-form
    cos_rep = np.tile(cos_il, (1, HQ))                      # [S, 512]
    sin_rep = np.tile(sin_il, (1, HQ))
    cos_t = np.ascontiguousarray(
        cos_rep.reshape(T, 128, E).transpose(1, 0, 2)).astype(bf)
    sin_t = np.ascontiguousarray(
        sin_rep.reshape(T, 128, E).transpose(1, 0, 2)).astype(bf)

    m2 = np.asarray(mask, dtype=np.float32)
    # per sk-chunk kc: the 128-wide partially-masked column block of the
    # transposed mask, as 0/1 visibility. mtd_t[p, kc, j] for global key
    # row 128*kc+p, query col 512*cq + 128*(kc-4*cq) + j.
    mtd_t = np.zeros((128, KC, 128), dtype=np.float32)
    for kc in range(KC):
        cq = kc // 4
        j0 = 128 * (kc - 4 * cq)
        qlo = 512 * cq + j0
        blk = m2[qlo:qlo + 128, 128 * kc:128 * (kc + 1)].T  # [sk 128, sq 128]
        mtd_t[:, kc, :] = np.where(blk > -1e29, 0.0, -3000.0)
    mtd_t = np.ascontiguousarray(mtd_t).astype(bf)

    wqf = np.asarray(wq, dtype=np.float32)
    wkf = np.asarray(wk, dtype=np.float32)
    wvf = np.asarray(wv, dtype=np.float32)
    wof = np.asarray(wo, dtype=np.float32)

    in_maps = []
    for c in range(N_CORES):
        wq_c = wqf[E * c:E * (c + 1), :]                    # [512, D]
        wqt = np.ascontiguousarray(
            wq_c.T.reshape(DC, 128, E).transpose(1, 0, 2)).astype(bf)
        wk_c = wkf[HD * c:HD * (c + 1), :]                  # [128, D]
        wv_c = wvf[HD * c:HD * (c + 1), :]
        wkv_c = np.concatenate([wk_c, wv_c], axis=0)        # [256, D]
        wkvt = np.ascontiguousarray(
            wkv_c.T.reshape(DC, 128, 2 * HD).transpose(1, 0, 2)).astype(bf)
        wo_c = wof[:, E * c:E * (c + 1)].T                  # [512 e, D]
        wo_t = np.ascontiguousarray(
            wo_c.reshape(HQ, 128, D).transpose(1, 0, 2)).astype(bf)
        in_maps.append({
            "xtt": xtt, "wqt": wqt, "wkvt": wkvt, "wo": wo_t,
            "mtd": mtd_t, "cos": cos_t, "sin": sin_t,
        })
    return in_maps


def kernel(x, wq, wk, wv, wo, freqs_cis, mask, start_pos=0):
    if "nc" not in _CACHE:
        _CACHE["nc"] = build_bass()
    nc = _CACHE["nc"]
    in_maps = _prep_inputs(x, wq, wk, wv, wo, freqs_cis, mask)
    res = run_bass_kernel_spmd(nc, in_maps, list(range(N_CORES)))
    total = np.zeros((S, D), dtype=np.float32)
    for c in range(N_CORES):
        total += res.results[c]["out"].astype(np.float32)
    return total.reshape(1, S, D)

